# revision 1
# baseline (speedup 1.0000x reference)
"""GCN (2-layer, mean/add/min/max aggregation) Trainium2 Bass kernel, 8 NeuronCores.

Sharding: nodes partitioned by destination across 8 cores (5000/core). Per core,
two phases of 2500 dests; per phase a private SBUF-resident bf16 table of the
needed source-node features (g = dinv * (h @ W.T)) is built with dma_gather
(int16 index range forces <=32768-row tables -> lo/hi split of the AllGathered
global table). Edge messages are gathered feature-major straight from SBUF
(dma_gather transpose=True), segment-reduced per 128-dest block with
tensor_reduce over a uniform padded slot axis (pad = duplicated self-edge,
exactly corrected for the sum), scaled by dinv[dest] (norm factorization
dinv[src]*dinv[dst] applied on the table side and after reduction), then
combined with the 512->128 matmul, bias and ReLU on PE/ACT. Final layer:
logits matmul + log_softmax on-chip.
"""
import sys

sys.path.insert(0, "/opt/trn_rl_repo")

import numpy as np
from contextlib import ExitStack

import concourse.bacc as bacc
import concourse.tile as tile
import concourse.mybir as mybir
from concourse import bass_utils

N = 40000
E = 640000
D = 128
NCLS = 40
CORES = 8
NPC = N // CORES            # 5000 nodes/core
PHASES = 2
DPP = NPC // PHASES         # 2500 dests/phase
BPP = (DPP + 127) // 128    # 20 blocks/phase
LPP = BPP * 128             # 2560 lanes/phase (incl pads)
NPADC = PHASES * LPP        # 5120 padded nodes/core
NG = CORES * NPADC          # 40960 global g rows
LO_SPLIT = 32768
MAX_GATHER = 8192
MSG_COLS = 6144


def _wrap_idx(idx):
    """int16 -> [128, n/16] wrapped (i -> [i%16, i//16]) and replicated x8."""
    idx = np.asarray(idx, dtype=np.int16)
    n = len(idx)
    assert n % 16 == 0
    cols = n // 16
    base = np.zeros((16, cols), dtype=np.int16)
    base[np.arange(n) % 16, np.arange(n) // 16] = idx
    return np.tile(base, (8, 1))


def _round_up(x, m):
    return (x + m - 1) // m * m


def _host_prep(x, edge_index):
    row = np.concatenate([np.asarray(edge_index[0]), np.arange(N, dtype=np.int64)])
    col = np.concatenate([np.asarray(edge_index[1]), np.arange(N, dtype=np.int64)])
    row = row.astype(np.int64)
    col = col.astype(np.int64)
    deg = np.bincount(col, minlength=N).astype(np.float64)
    dinv = deg ** -0.5
    invdeg = 1.0 / deg

    # per-core, per-phase degree-sorted dest order
    order = np.zeros((CORES, PHASES, LPP), dtype=np.int64)  # local dest in [0,2500) or -1
    perm_cols = np.full((CORES, NPADC), -1, dtype=np.int64)  # col -> local node id (0..4999) or -1
    col_of_local = np.zeros((CORES, NPC), dtype=np.int64)
    for c in range(CORES):
        degs_c = deg[c * NPC:(c + 1) * NPC]
        for p in range(PHASES):
            degs = degs_c[p * DPP:(p + 1) * DPP]
            o = np.argsort(-degs, kind="stable")
            ordp = np.full(LPP, -1, dtype=np.int64)
            ordp[:DPP] = o
            order[c, p] = ordp
            loc = p * DPP + o
            cols = p * LPP + np.arange(DPP)
            perm_cols[c, p * LPP:p * LPP + DPP] = loc
            col_of_local[c, loc] = cols
    gpos = np.zeros(N, dtype=np.int64)
    for c in range(CORES):
        gpos[c * NPC:(c + 1) * NPC] = c * NPADC + col_of_local[c]

    # global uniform slot counts per (phase, block)
    S = np.zeros((PHASES, BPP), dtype=np.int64)
    for c in range(CORES):
        degs_c = deg[c * NPC:(c + 1) * NPC]
        for p in range(PHASES):
            for b in range(BPP):
                lanes = order[c, p, b * 128:(b + 1) * 128]
                real = lanes[lanes >= 0]
                if len(real):
                    S[p, b] = max(S[p, b], int(degs_c[p * DPP + real].max()))
    S = np.maximum(S, 1)
    P0 = np.zeros((PHASES, BPP + 1), dtype=np.int64)
    for p in range(PHASES):
        P0[p, 1:] = np.cumsum(128 * S[p])
    LPH = [int(P0[p, -1]) for p in range(PHASES)]

    # per-core edge grouping (sorted by dest, self-edge first)
    core_edges = []
    for c in range(CORES):
        sel = (col >= c * NPC) & (col < (c + 1) * NPC)
        ec = col[sel] - c * NPC
        er = row[sel]
        not_self = (er != col[sel]).astype(np.int64)
        sidx = np.lexsort((gpos[er], not_self, ec))
        ec, er = ec[sidx], er[sidx]
        cnt = np.bincount(ec, minlength=NPC)
        off = np.zeros(NPC + 1, dtype=np.int64)
        off[1:] = np.cumsum(cnt)
        core_edges.append((er, off, cnt))

    # private tables (lo/hi split of gpos space), global padded sizes
    lo_lists, hi_lists = {}, {}
    lo_max = hi_max = 0
    for c in range(CORES):
        er, off, cnt = core_edges[c]
        for p in range(PHASES):
            e0, e1 = off[p * DPP], off[(p + 1) * DPP]
            used = np.unique(gpos[er[e0:e1]])
            lo = used[used < LO_SPLIT]
            hi = used[used >= LO_SPLIT]
            lo_lists[c, p] = lo
            hi_lists[c, p] = hi
            lo_max, hi_max = max(lo_max, len(lo)), max(hi_max, len(hi))
    LO_PAD = _round_up(max(lo_max, 128), 128)
    HI_PAD = _round_up(max(hi_max, 128), 128)
    TOK = LO_PAD + HI_PAD

    # per-core arrays
    per_core = []
    for c in range(CORES):
        er, off, cnt = core_edges[c]
        ed_tok = [np.zeros(LPH[p], dtype=np.int64) for p in range(PHASES)]
        npad_l = np.zeros(NPADC, dtype=np.float64)
        bidx = {}
        for p in range(PHASES):
            lo, hi = lo_lists[c, p], hi_lists[c, p]
            tok_map = np.full(NG, -1, dtype=np.int64)
            tok_map[lo] = np.arange(len(lo))
            tok_map[hi] = LO_PAD + np.arange(len(hi))
            lo_pad = np.zeros(LO_PAD, dtype=np.int64)
            lo_pad[:len(lo)] = lo
            hi_pad = np.zeros(HI_PAD, dtype=np.int64)
            hi_pad[:len(hi)] = hi - LO_SPLIT
            bidx[p] = (lo_pad, hi_pad)
            for b in range(BPP):
                sb = S[p, b]
                base_b = P0[p, b]
                for l in range(128):
                    colid = p * LPP + b * 128 + l
                    dl = order[c, p, b * 128 + l]
                    base = base_b + l * sb
                    if dl < 0:
                        npad_l[colid] = sb
                        continue  # tokens stay 0
                    loc = p * DPP + dl
                    dg = int(cnt[loc])
                    toks = tok_map[gpos[er[off[loc]:off[loc] + dg]]]
                    ed_tok[p][base:base + dg] = toks
                    ed_tok[p][base + dg:base + sb] = toks[0]
                    npad_l[colid] = sb - dg
        nodes = perm_cols[c]
        real = nodes >= 0
        gl = np.where(real, c * NPC + nodes, 0)
        xT = np.zeros((128, NPADC), dtype=np.float32)
        xp = np.zeros((NPADC, D), dtype=np.float32)
        xp[real] = np.asarray(x)[gl[real]]
        xT = np.ascontiguousarray(xp.T)
        dinv_l = np.where(real, dinv[gl], 1.0)
        dinvdeg_l = np.where(real, (dinv * invdeg)[gl], 1.0)
        per_core.append(dict(
            xT=xT,
            dinv_scale=np.ascontiguousarray(
                dinv_l.reshape(NPADC // 128, 128).T).astype(np.float32),
            dinv_b=np.broadcast_to(dinv_l, (128, NPADC)).astype(np.float32).copy(),
            dinvdeg_b=np.broadcast_to(dinvdeg_l, (128, NPADC)).astype(np.float32).copy(),
            npad_b=np.broadcast_to(npad_l, (128, NPADC)).astype(np.float32).copy(),
            eidx0=_wrap_idx(ed_tok[0]), eidx1=_wrap_idx(ed_tok[1]),
            blo0=_wrap_idx(bidx[0][0]), bhi0=_wrap_idx(bidx[0][1]),
            blo1=_wrap_idx(bidx[1][0]), bhi1=_wrap_idx(bidx[1][1]),
            real=real, gl=gl,
        ))
    meta = dict(S=S, P0=P0, LPH=LPH, LO_PAD=LO_PAD, HI_PAD=HI_PAD, TOK=TOK)
    return per_core, meta


def _build_program(meta):
    S, P0, LPH = meta["S"], meta["P0"], meta["LPH"]
    LO_PAD, HI_PAD, TOK = meta["LO_PAD"], meta["HI_PAD"], meta["TOK"]
    TOKB = TOK // 128
    f32, bf16, i16 = mybir.dt.float32, mybir.dt.bfloat16, mybir.dt.int16
    AX = mybir.AxisListType.X
    OP = mybir.AluOpType
    AF = mybir.ActivationFunctionType

    nc = bacc.Bacc("TRN2", target_bir_lowering=False, debug=False,
                   num_devices=CORES)
    t_xT = nc.dram_tensor("xT", [128, NPADC], f32, kind="ExternalInput")
    t_w = [nc.dram_tensor(f"W{l}T", [128, 128], f32, kind="ExternalInput") for l in range(2)]
    t_c = [nc.dram_tensor(f"C{l}T", [4, 128, 128], f32, kind="ExternalInput") for l in range(2)]
    t_b = [nc.dram_tensor(f"b{l}", [128, 1], f32, kind="ExternalInput") for l in range(2)]
    t_wout = nc.dram_tensor("WoutT", [128, NCLS], f32, kind="ExternalInput")
    t_bout = nc.dram_tensor("boutb", [128, NCLS], f32, kind="ExternalInput")
    t_dsc = nc.dram_tensor("dinv_scale", [128, NPADC // 128], f32, kind="ExternalInput")
    t_dinvb = nc.dram_tensor("dinv_b", [128, NPADC], f32, kind="ExternalInput")
    t_ddegb = nc.dram_tensor("dinvdeg_b", [128, NPADC], f32, kind="ExternalInput")
    t_npadb = nc.dram_tensor("npad_b", [128, NPADC], f32, kind="ExternalInput")
    t_eidx = [nc.dram_tensor(f"eidx{p}", [128, LPH[p] // 16], i16, kind="ExternalInput")
              for p in range(PHASES)]
    t_blo = [nc.dram_tensor(f"blo{p}", [128, LO_PAD // 16], i16, kind="ExternalInput")
             for p in range(PHASES)]
    t_bhi = [nc.dram_tensor(f"bhi{p}", [128, HI_PAD // 16], i16, kind="ExternalInput")
             for p in range(PHASES)]
    t_out = nc.dram_tensor("out", [NPADC, NCLS], f32, kind="ExternalOutput")
    t_gsh = nc.dram_tensor("gsh", [NPADC, D], bf16, kind="Internal")
    t_gfull = nc.dram_tensor("gfull", [NG, D], bf16, kind="Internal")

    NCH = NPADC // 128  # 40 node chunks per core

    with tile.TileContext(nc) as tc, ExitStack() as ctx:
        sb = ctx.enter_context(tc.tile_pool(name="sb", bufs=1))
        lhsp = ctx.enter_context(tc.tile_pool(name="lhsp", bufs=3))
        msgp = ctx.enter_context(tc.tile_pool(name="msgp", bufs=3))
        pg = ctx.enter_context(tc.tile_pool(name="pg", bufs=2, space="PSUM"))
        pc = ctx.enter_context(tc.tile_pool(name="pc", bufs=2, space="PSUM"))

        hT = sb.tile([128, NPADC], f32, tag="hT")
        dsc = sb.tile([128, NCH], f32, tag="dsc")
        nc.sync.dma_start(dsc[:], t_dsc.ap())

        for l in range(2):
            wt = sb.tile([128, 128], f32, tag="wt")
            ct = sb.tile([128, 4, 128], f32, tag="ct")
            bt = sb.tile([128, 1], f32, tag="bt")
            nc.sync.dma_start(wt[:], t_w[l].ap())
            nc.sync.dma_start(ct[:], t_c[l].ap().rearrange("k p f -> p k f"))
            nc.sync.dma_start(bt[:], t_b[l].ap())

            # ---- A: g shard = dinv * (in @ W.T), row-major bf16, DMA to gsh
            for j in range(NCH):
                if l == 0:
                    lhs = lhsp.tile([128, 128], f32, tag="lhs")
                    nc.sync.dma_start(lhs[:], t_xT.ap()[:, j * 128:(j + 1) * 128])
                    lhs_ap = lhs[:]
                else:
                    lhs_ap = hT[:, j * 128:(j + 1) * 128]
                ps = pg.tile([128, 128], f32, tag="ps_g")
                nc.tensor.matmul(ps[:], lhsT=lhs_ap, rhs=wt[:], start=True, stop=True)
                gt = lhsp.tile([128, 128], bf16, tag="gt")
                nc.scalar.activation(gt[:], ps[:], AF.Copy, scale=dsc[:, j:j + 1])
                nc.sync.dma_start(
                    t_gsh.ap().rearrange("(a p) d -> p a d", p=128)[:, j, :], gt[:])

            nc.gpsimd.collective_compute(
                "AllGather", OP.bypass, replica_groups=[list(range(CORES))],
                ins=[t_gsh.ap()], outs=[t_gfull.ap()])

            for p in range(PHASES):
                # ---- B: private table build (lo/hi ranges, <=8K idx chunks)
                table = sb.tile([128, TOKB, 128], bf16, tag="table")
                blo = sb.tile([128, LO_PAD // 16], i16, tag="blo")
                bhi = sb.tile([128, HI_PAD // 16], i16, tag="bhi")
                nc.sync.dma_start(blo[:], t_blo[p].ap())
                nc.sync.dma_start(bhi[:], t_bhi[p].ap())
                for base, npd, idx_t, r0, r1 in (
                        (0, LO_PAD, blo, 0, LO_SPLIT),
                        (LO_PAD, HI_PAD, bhi, LO_SPLIT, NG)):
                    for c0 in range(0, npd, MAX_GATHER):
                        cn = min(MAX_GATHER, npd - c0)
                        nc.gpsimd.dma_gather(
                            out_ap=table[:, (base + c0) // 128:(base + c0 + cn) // 128, :],
                            in_ap=t_gfull.ap()[r0:r1, :],
                            idxs_ap=idx_t[:, c0 // 16:(c0 + cn) // 16],
                            num_idxs=cn, num_idxs_reg=cn, elem_size=D,
                            single_packet=False)

                eix = sb.tile([128, LPH[p] // 16], i16, tag="eix")
                nc.sync.dma_start(eix[:], t_eidx[p].ap())
                dinvb = sb.tile([128, LPP], f32, tag="dinvb")
                ddegb = sb.tile([128, LPP], f32, tag="ddegb")
                npadb = sb.tile([128, LPP], f32, tag="npadb")
                nc.sync.dma_start(dinvb[:], t_dinvb.ap()[:, p * LPP:(p + 1) * LPP])
                nc.sync.dma_start(ddegb[:], t_ddegb.ap()[:, p * LPP:(p + 1) * LPP])
                nc.sync.dma_start(npadb[:], t_npadb.ap()[:, p * LPP:(p + 1) * LPP])
                stat_add = sb.tile([128, LPP], f32, tag="stat_add")
                stat_mn = sb.tile([128, LPP], f32, tag="stat_mn")
                stat_mx = sb.tile([128, LPP], f32, tag="stat_mx")
                stat_mean = sb.tile([128, LPP], f32, tag="npadb")

                # ---- C/D: edge gather chunks + per-block reduces
                chunks = []
                cur, cur_cols = [], 0
                for b in range(BPP):
                    w = 128 * int(S[p, b])
                    if cur and cur_cols + w > MSG_COLS:
                        chunks.append(cur)
                        cur, cur_cols = [], 0
                    cur.append(b)
                    cur_cols += w
                if cur:
                    chunks.append(cur)
                for ch in chunks:
                    q0 = int(P0[p, ch[0]])
                    qn = int(P0[p, ch[-1] + 1]) - q0
                    msg = msgp.tile([128, 1, MSG_COLS], bf16, tag="msg")
                    nc.gpsimd.dma_gather(
                        out_ap=msg[:, :, :qn], in_ap=table[:],
                        idxs_ap=eix[:, q0 // 16:(q0 + qn) // 16],
                        num_idxs=qn, num_idxs_reg=qn, elem_size=D,
                        transpose=True, sbuf_tokens_per_rank=128,
                        sbuf_free_dim_per_rank=D * 2, single_packet=False)
                    for b in ch:
                        sbl = int(S[p, b])
                        cb = int(P0[p, b]) - q0
                        view = msg[:, 0, cb:cb + 128 * sbl].rearrange(
                            "p (l s) -> p l s", s=sbl)
                        dsl = slice(b * 128, (b + 1) * 128)
                        nc.vector.tensor_reduce(
                            out=stat_add[:, dsl], in_=view, axis=AX, op=OP.add)
                        nc.vector.tensor_reduce(
                            out=stat_mn[:, dsl], in_=view, axis=AX, op=OP.min)
                        nc.vector.tensor_reduce(
                            out=stat_mx[:, dsl], in_=view, axis=AX, op=OP.max)
                        tmp = lhsp.tile([128, 128], f32, tag="tmp")
                        nc.vector.tensor_tensor(
                            out=tmp[:], in0=view[:, :, 0], in1=npadb[:, dsl],
                            op=OP.mult)
                        nc.vector.tensor_tensor(
                            out=stat_add[:, dsl], in0=stat_add[:, dsl],
                            in1=tmp[:], op=OP.subtract)

                # ---- scale stats
                nc.vector.tensor_tensor(out=stat_mean[:], in0=stat_add[:],
                                        in1=ddegb[:], op=OP.mult)
                nc.vector.tensor_tensor(out=stat_add[:], in0=stat_add[:],
                                        in1=dinvb[:], op=OP.mult)
                nc.vector.tensor_tensor(out=stat_mn[:], in0=stat_mn[:],
                                        in1=dinvb[:], op=OP.mult)
                nc.vector.tensor_tensor(out=stat_mx[:], in0=stat_mx[:],
                                        in1=dinvb[:], op=OP.mult)

                # ---- E: combine matmuls (feature-major h out) + bias + relu
                for g in range(LPP // 512):
                    psc = pc.tile([128, 512], f32, tag="ps_cmb")
                    for k, st in enumerate((stat_mean, stat_add, stat_mn, stat_mx)):
                        nc.tensor.matmul(
                            psc[:], lhsT=ct[:, k, :],
                            rhs=st[:, g * 512:(g + 1) * 512],
                            start=(k == 0), stop=(k == 3))
                    nc.scalar.activation(
                        hT[:, p * LPP + g * 512:p * LPP + (g + 1) * 512],
                        psc[:], AF.Relu, bias=bt[:], scale=1.0)

        # ---- logits + log_softmax
        wout = sb.tile([128, NCLS], f32, tag="wout")
        bout = sb.tile([128, NCLS], f32, tag="bout")
        nc.sync.dma_start(wout[:], t_wout.ap())
        nc.sync.dma_start(bout[:], t_bout.ap())
        for j in range(NCH):
            ps = pg.tile([128, NCLS], f32, tag="ps_lg")
            nc.tensor.matmul(ps[:], lhsT=hT[:, j * 128:(j + 1) * 128],
                             rhs=wout[:], start=True, stop=True)
            lg = lhsp.tile([128, NCLS], f32, tag="lg")
            nc.vector.tensor_tensor(out=lg[:], in0=ps[:], in1=bout[:], op=OP.add)
            mx = lhsp.tile([128, 1], f32, tag="mx")
            nc.vector.tensor_reduce(out=mx[:], in_=lg[:], axis=AX, op=OP.max)
            nc.vector.tensor_scalar_sub(lg[:], lg[:], mx[:])
            ex = lhsp.tile([128, NCLS], f32, tag="ex")
            nc.scalar.activation(ex[:], lg[:], AF.Exp)
            se = lhsp.tile([128, 1], f32, tag="se")
            nc.vector.tensor_reduce(out=se[:], in_=ex[:], axis=AX, op=OP.add)
            ls = lhsp.tile([128, 1], f32, tag="ls")
            nc.scalar.activation(ls[:], se[:], AF.Ln)
            nc.vector.tensor_scalar_sub(lg[:], lg[:], ls[:])
            nc.sync.dma_start(
                t_out.ap().rearrange("(a p) n -> p a n", p=128)[:, j, :], lg[:])

    nc.compile()
    return nc


_CACHE = {}


def kernel(x, edge_index, W0, C0, b0, W1, C1, b1, Wout, bout,
           trace=False, _want_results=False):
    x = np.asarray(x, dtype=np.float32)
    per_core, meta = _host_prep(x, edge_index)
    key = (meta["TOK"], tuple(meta["LPH"]))
    if key not in _CACHE:
        _CACHE[key] = _build_program(meta)
    nc = _CACHE[key]

    shared = dict(
        W0T=np.ascontiguousarray(np.asarray(W0, np.float32).T),
        W1T=np.ascontiguousarray(np.asarray(W1, np.float32).T),
        C0T=np.ascontiguousarray(np.asarray(C0, np.float32).T).reshape(4, 128, 128),
        C1T=np.ascontiguousarray(np.asarray(C1, np.float32).T).reshape(4, 128, 128),
        b0=np.asarray(b0, np.float32).reshape(128, 1),
        b1=np.asarray(b1, np.float32).reshape(128, 1),
        WoutT=np.ascontiguousarray(np.asarray(Wout, np.float32).T),
        boutb=np.broadcast_to(np.asarray(bout, np.float32), (128, NCLS)).copy(),
    )
    in_maps = []
    for c in range(CORES):
        d = per_core[c]
        m = dict(shared)
        m.update(xT=d["xT"], dinv_scale=d["dinv_scale"], dinv_b=d["dinv_b"],
                 dinvdeg_b=d["dinvdeg_b"], npad_b=d["npad_b"],
                 eidx0=d["eidx0"], eidx1=d["eidx1"],
                 blo0=d["blo0"], bhi0=d["bhi0"], blo1=d["blo1"], bhi1=d["bhi1"])
        in_maps.append(m)

    res = bass_utils.run_bass_kernel_spmd(
        nc, in_maps, core_ids=list(range(CORES)), trace=trace)

    out = np.zeros((N, NCLS), dtype=np.float32)
    for c in range(CORES):
        o = res.results[c]["out"]
        d = per_core[c]
        out[d["gl"][d["real"]]] = o[d["real"]]
    if _want_results:
        return out, res
    return out



# revision 2
# speedup vs baseline: 1.5751x; 1.5751x over previous
"""GCN (2-layer, mean/add/min/max aggregation) Trainium2 Bass kernel, 8 NeuronCores.

v2: table-free edge gather. Nodes partitioned by destination across 8 cores
(5000/core, one degree-sorted phase of 40 x 128 lanes). Per layer each core
computes g = dinv * (h @ W.T) for its shard (bf16), AllGathers to a global
DRAM table gfull [40960, 128]. Edge messages are gathered feature-major
straight from DRAM (dma_gather transpose=True) -- no SBUF staging table.
The int16 gather-index limit (<32768) is handled with two overlapping source
windows, A=[0,32768) and B=[8192,40960): each dest's edges are split between
two message buffers (balanced ~deg/2 per side inside each 128-lane block to
keep slot padding low; window overlap [8192,32768) gives free choice for 60%
of edges). Per 128-dest block both sides are segment-reduced (add in f32 with
exact pad correction, min/max in bf16) and the sides combined with three
full-width vector ops. Stats are scaled by dinv[dest] / deg and fed to the
512->128 combine matmul in bf16, with bias+ReLU on ACT. Final layer: logits
matmul batched 4 node-chunks per PSUM group + constant-shift log_softmax.
"""
import sys

sys.path.insert(0, "/opt/trn_rl_repo")

import numpy as np
from contextlib import ExitStack

import concourse.bacc as bacc
import concourse.tile as tile
import concourse.mybir as mybir
from concourse import bass_utils

N = 40000
E = 640000
D = 128
NCLS = 40
CORES = 8
NPC = N // CORES            # 5000 nodes/core
NPADC = 5120                # padded nodes/core (40 blocks of 128 lanes)
BLK = NPADC // 128          # 40 blocks
NG = CORES * NPADC          # 40960 global g rows
WIN = 32768                 # int16 window size
BOFF = NG - WIN             # 8192: window B covers [8192, 40960)
MSG_COLS = 6144


def _wrap_idx(idx):
    """int16 -> [128, n/16] wrapped (i -> [i%16, i//16]) and replicated x8."""
    idx = np.asarray(idx, dtype=np.int16)
    n = len(idx)
    assert n % 16 == 0
    cols = n // 16
    base = np.zeros((16, cols), dtype=np.int16)
    base[np.arange(n) % 16, np.arange(n) // 16] = idx
    return np.tile(base, (8, 1))


def _host_prep(x, edge_index):
    row = np.concatenate([np.asarray(edge_index[0]), np.arange(N, dtype=np.int64)])
    col = np.concatenate([np.asarray(edge_index[1]), np.arange(N, dtype=np.int64)])
    row = row.astype(np.int64)
    col = col.astype(np.int64)
    deg = np.bincount(col, minlength=N).astype(np.float64)
    dinv = np.where(deg > 0, deg ** -0.5, 0.0)
    ddeg = np.where(deg > 0, dinv / np.maximum(deg, 1.0), 0.0)

    # per-core degree-sorted lane order; gpos = global row in gfull
    lane_of_node = np.zeros(N, dtype=np.int64)   # node -> lane (0..5119)
    node_of_lane = np.full((CORES, NPADC), -1, dtype=np.int64)
    for c in range(CORES):
        degs_c = deg[c * NPC:(c + 1) * NPC]
        o = np.argsort(-degs_c, kind="stable")
        lane_of_node[c * NPC + o] = np.arange(NPC)
        node_of_lane[c, :NPC] = c * NPC + o
    gpos = np.zeros(N, dtype=np.int64)
    for c in range(CORES):
        gpos[c * NPC:(c + 1) * NPC] = c * NPADC + lane_of_node[c * NPC:(c + 1) * NPC]

    # per-core edge lists sorted by (lane, side-category)
    per_core_edges = []
    nA_all = np.zeros((CORES, NPADC), dtype=np.int64)
    nB_all = np.zeros((CORES, NPADC), dtype=np.int64)
    for c in range(CORES):
        sel = (col >= c * NPC) & (col < (c + 1) * NPC)
        lanes = lane_of_node[col[sel]]
        gp = gpos[row[sel]]
        cat = np.ones(len(gp), dtype=np.int64)          # free
        cat[gp < BOFF] = 0                              # must-A
        cat[gp >= WIN] = 2                              # must-B
        sidx = np.lexsort((cat, lanes))
        lanes, gp, cat = lanes[sidx], gp[sidx], cat[sidx]
        cnt = np.bincount(lanes, minlength=NPADC)
        off = np.zeros(NPADC + 1, dtype=np.int64)
        off[1:] = np.cumsum(cnt)
        mA = np.bincount(lanes[cat == 0], minlength=NPADC)
        mB = np.bincount(lanes[cat == 2], minlength=NPADC)
        t = (cnt + 1) // 2
        nA = np.minimum(np.maximum(t, mA), cnt - mB)
        nB = cnt - nA
        # real dests must populate both sides (deg>=2 and window overlap make
        # this hold for every graph we expect; verified here)
        real = cnt > 0
        bad = real & ((nA == 0) | (nB == 0)) & (cnt >= 2)
        assert not bad.any(), "dest with an unpopulatable gather side"
        deg1 = real & (cnt == 1)
        assert not deg1.any(), "isolated dest (deg==1) needs mask handling"
        per_core_edges.append((lanes, gp, off, cnt))
        nA_all[c], nB_all[c] = nA, nB

    # global uniform slot counts per (block, side)
    SA = np.maximum(nA_all.reshape(CORES, BLK, 128).max(axis=(0, 2)), 1)
    SB = np.maximum(nB_all.reshape(CORES, BLK, 128).max(axis=(0, 2)), 1)
    PA = np.zeros(BLK + 1, dtype=np.int64)
    PA[1:] = np.cumsum(128 * SA)
    PB = np.zeros(BLK + 1, dtype=np.int64)
    PB[1:] = np.cumsum(128 * SB)
    colsA, colsB = int(PA[-1]), int(PB[-1])

    per_core = []
    for c in range(CORES):
        lanes, gp, off, cnt = per_core_edges[c]
        nA, nB = nA_all[c], nB_all[c]
        blk = np.arange(NPADC) // 128
        lane_in_blk = np.arange(NPADC) % 128
        baseA = PA[blk] + lane_in_blk * SA[blk]
        baseB = PB[blk] + lane_in_blk * SB[blk]

        rank = np.arange(len(lanes)) - off[lanes]
        isA = rank < nA[lanes]
        posA = baseA[lanes] + rank
        posB = baseB[lanes] + (rank - nA[lanes])
        tokA_real = gp[isA]
        tokB_real = gp[~isA] - BOFF
        assert len(tokA_real) == 0 or (0 <= tokA_real.min() and tokA_real.max() < WIN)
        assert len(tokB_real) == 0 or (0 <= tokB_real.min() and tokB_real.max() < WIN)

        # slot-0 token per lane (pad value source); 0 for empty lanes
        tok0A = np.zeros(NPADC, dtype=np.int64)
        hasA = nA > 0
        tok0A[lanes[isA & (rank == 0)]] = gp[isA & (rank == 0)]
        tok0B = np.zeros(NPADC, dtype=np.int64)
        firstB = (~isA) & (rank == nA[lanes])
        tok0B[lanes[firstB]] = gp[firstB] - BOFF

        edA = np.zeros(colsA, dtype=np.int64)
        edB = np.zeros(colsB, dtype=np.int64)
        for b in range(BLK):
            lv = slice(b * 128, (b + 1) * 128)
            edA[PA[b]:PA[b + 1]] = np.repeat(tok0A[lv], SA[b])
            edB[PB[b]:PB[b + 1]] = np.repeat(tok0B[lv], SB[b])
        edA[posA[isA]] = tokA_real
        edB[posB[~isA]] = tokB_real

        npadA = (SA[blk] - nA).astype(np.float64)
        npadB = (SB[blk] - nB).astype(np.float64)

        nodes = node_of_lane[c]
        real = nodes >= 0
        gl = np.where(real, nodes, 0)
        xp = np.zeros((NPADC, D), dtype=np.float32)
        xp[real] = np.asarray(x)[gl[real]]
        xT = np.ascontiguousarray(xp.T)
        dinv_l = np.where(real, dinv[gl], 0.0)
        ddeg_l = np.where(real, ddeg[gl], 0.0)

        bf = np.float32  # broadcast tensors stored f32->bf16 by bass? keep bf16
        per_core.append(dict(
            xT=xT,
            dinv_scale=np.ascontiguousarray(
                dinv_l.reshape(BLK, 128).T).astype(np.float32),
            dinvb=np.broadcast_to(dinv_l, (128, NPADC)).astype(np.float32).copy(),
            ddegb=np.broadcast_to(ddeg_l, (128, NPADC)).astype(np.float32).copy(),
            npadbA=np.broadcast_to(npadA, (128, NPADC)).astype(np.float32).copy(),
            npadbB=np.broadcast_to(npadB, (128, NPADC)).astype(np.float32).copy(),
            eidxA=_wrap_idx(edA), eidxB=_wrap_idx(edB),
            real=real, gl=gl,
        ))
    meta = dict(SA=SA, SB=SB, PA=PA, PB=PB, colsA=colsA, colsB=colsB)
    return per_core, meta


def _chunks(S, P, max_cols):
    out, cur, cur_cols = [], [], 0
    for b in range(BLK):
        w = 128 * int(S[b])
        if cur and cur_cols + w > max_cols:
            out.append(cur)
            cur, cur_cols = [], 0
        cur.append(b)
        cur_cols += w
    if cur:
        out.append(cur)
    return out


def _build_program(meta):
    SA, SB, PA, PB = meta["SA"], meta["SB"], meta["PA"], meta["PB"]
    colsA, colsB = meta["colsA"], meta["colsB"]
    f32, bf16, i16 = mybir.dt.float32, mybir.dt.bfloat16, mybir.dt.int16
    AX = mybir.AxisListType.X
    OP = mybir.AluOpType
    AF = mybir.ActivationFunctionType

    nc = bacc.Bacc("TRN2", target_bir_lowering=False, debug=False,
                   num_devices=CORES)
    t_xT = nc.dram_tensor("xT", [128, NPADC], f32, kind="ExternalInput")
    t_w = [nc.dram_tensor(f"W{l}T", [128, 128], f32, kind="ExternalInput") for l in range(2)]
    t_c = [nc.dram_tensor(f"C{l}T", [4, 128, 128], bf16, kind="ExternalInput") for l in range(2)]
    t_b = [nc.dram_tensor(f"b{l}", [128, 1], f32, kind="ExternalInput") for l in range(2)]
    t_wout = nc.dram_tensor("WoutT", [128, NCLS], f32, kind="ExternalInput")
    t_bout4 = nc.dram_tensor("bout4", [128, 4, NCLS], f32, kind="ExternalInput")
    t_dsc = nc.dram_tensor("dinv_scale", [128, BLK], f32, kind="ExternalInput")
    t_dinvb = nc.dram_tensor("dinvb", [128, NPADC], bf16, kind="ExternalInput")
    t_ddegb = nc.dram_tensor("ddegb", [128, NPADC], bf16, kind="ExternalInput")
    t_npadA = nc.dram_tensor("npadbA", [128, NPADC], bf16, kind="ExternalInput")
    t_npadB = nc.dram_tensor("npadbB", [128, NPADC], bf16, kind="ExternalInput")
    t_eidxA = nc.dram_tensor("eidxA", [128, colsA // 16], i16, kind="ExternalInput")
    t_eidxB = nc.dram_tensor("eidxB", [128, colsB // 16], i16, kind="ExternalInput")
    t_out = nc.dram_tensor("out", [NPADC, NCLS], f32, kind="ExternalOutput")
    t_gsh = nc.dram_tensor("gsh", [NPADC, D], bf16, kind="Internal")
    t_gfull = nc.dram_tensor("gfull", [NG, D], bf16, kind="Internal")

    NCH = NPADC // 128  # 40 node chunks per core
    chA = _chunks(SA, PA, MSG_COLS)
    chB = _chunks(SB, PB, MSG_COLS)

    with tile.TileContext(nc) as tc, ExitStack() as ctx:
        sb = ctx.enter_context(tc.tile_pool(name="sb", bufs=1))
        lhsp = ctx.enter_context(tc.tile_pool(name="lhsp", bufs=3))
        msgp = ctx.enter_context(tc.tile_pool(name="msgp", bufs=3))
        eixp = ctx.enter_context(tc.tile_pool(name="eixp", bufs=2))
        rhp = ctx.enter_context(tc.tile_pool(name="rhp", bufs=2))
        pg = ctx.enter_context(tc.tile_pool(name="pg", bufs=2, space="PSUM"))
        pc = ctx.enter_context(tc.tile_pool(name="pc", bufs=2, space="PSUM"))
        plg = ctx.enter_context(tc.tile_pool(name="plg", bufs=2, space="PSUM"))

        hT = sb.tile([128, NPADC], f32, tag="hT")
        dsc = sb.tile([128, NCH], f32, tag="dsc")
        dinvb = sb.tile([128, NPADC], bf16, tag="dinvb")
        ddegb = sb.tile([128, NPADC], bf16, tag="ddegb")
        npadA = sb.tile([128, NPADC], bf16, tag="npadA")
        npadB = sb.tile([128, NPADC], bf16, tag="npadB")
        nc.sync.dma_start(dsc[:], t_dsc.ap())
        nc.sync.dma_start(dinvb[:], t_dinvb.ap())
        nc.sync.dma_start(ddegb[:], t_ddegb.ap())
        nc.sync.dma_start(npadA[:], t_npadA.ap())
        nc.sync.dma_start(npadB[:], t_npadB.ap())

        stA_add = sb.tile([128, NPADC], f32, tag="stA_add")
        stB_add = sb.tile([128, NPADC], f32, tag="stB_add")
        stA_mn = sb.tile([128, NPADC], bf16, tag="stA_mn")
        stB_mn = sb.tile([128, NPADC], bf16, tag="stB_mn")
        stA_mx = sb.tile([128, NPADC], bf16, tag="stA_mx")
        stB_mx = sb.tile([128, NPADC], bf16, tag="stB_mx")

        for l in range(2):
            wt = sb.tile([128, 128], f32, tag="wt")
            ct = sb.tile([128, 4, 128], bf16, tag="ct")
            bt = sb.tile([128, 1], f32, tag="bt")
            nc.sync.dma_start(wt[:], t_w[l].ap())
            nc.sync.dma_start(ct[:], t_c[l].ap().rearrange("k p f -> p k f"))
            nc.sync.dma_start(bt[:], t_b[l].ap())

            # ---- A: g shard = dinv * (in @ W.T), row-major bf16, DMA to gsh
            for j in range(NCH):
                if l == 0:
                    lhs = lhsp.tile([128, 128], f32, tag="lhs")
                    nc.sync.dma_start(lhs[:], t_xT.ap()[:, j * 128:(j + 1) * 128])
                    lhs_ap = lhs[:]
                else:
                    lhs_ap = hT[:, j * 128:(j + 1) * 128]
                ps = pg.tile([128, 128], f32, tag="ps_g")
                nc.tensor.matmul(ps[:], lhsT=lhs_ap, rhs=wt[:], start=True, stop=True)
                gt = lhsp.tile([128, 128], bf16, tag="gt")
                nc.scalar.activation(gt[:], ps[:], AF.Copy, scale=dsc[:, j:j + 1])
                nc.sync.dma_start(
                    t_gsh.ap().rearrange("(a p) d -> p a d", p=128)[:, j, :], gt[:])

            nc.gpsimd.collective_compute(
                "AllGather", OP.bypass, replica_groups=[list(range(CORES))],
                ins=[t_gsh.ap()], outs=[t_gfull.ap()])

            # ---- C/D: direct-DRAM edge gathers + per-block reduces
            for side, (chs, S, P, t_eidx, src_lo, src_hi, st_add, st_mn, st_mx, npadS) in enumerate((
                    (chA, SA, PA, t_eidxA, 0, WIN, stA_add, stA_mn, stA_mx, npadA),
                    (chB, SB, PB, t_eidxB, BOFF, NG, stB_add, stB_mn, stB_mx, npadB))):
                for ch in chs:
                    q0 = int(P[ch[0]])
                    qn = int(P[ch[-1] + 1]) - q0
                    eix = eixp.tile([128, MSG_COLS // 16], i16, tag="eix")
                    nc.sync.dma_start(eix[:, :qn // 16],
                                      t_eidx.ap()[:, q0 // 16:(q0 + qn) // 16])
                    msg = msgp.tile([128, 1, MSG_COLS], bf16, tag="msg")
                    nc.gpsimd.dma_gather(
                        out_ap=msg[:, :, :qn],
                        in_ap=t_gfull.ap()[src_lo:src_hi, :],
                        idxs_ap=eix[:, :qn // 16],
                        num_idxs=qn, num_idxs_reg=qn, elem_size=D,
                        transpose=True, single_packet=False)
                    for b in ch:
                        sbl = int(S[b])
                        cb = int(P[b]) - q0
                        view = msg[:, 0, cb:cb + 128 * sbl].rearrange(
                            "p (l s) -> p l s", s=sbl)
                        dsl = slice(b * 128, (b + 1) * 128)
                        nc.vector.tensor_reduce(
                            out=st_add[:, dsl], in_=view, axis=AX, op=OP.add)
                        nc.vector.tensor_reduce(
                            out=st_mn[:, dsl], in_=view, axis=AX, op=OP.min)
                        nc.vector.tensor_reduce(
                            out=st_mx[:, dsl], in_=view, axis=AX, op=OP.max)
                        tmp = lhsp.tile([128, 128], f32, tag="tmp")
                        nc.vector.tensor_tensor(
                            out=tmp[:], in0=view[:, :, 0], in1=npadS[:, dsl],
                            op=OP.mult)
                        nc.vector.tensor_tensor(
                            out=st_add[:, dsl], in0=st_add[:, dsl],
                            in1=tmp[:], op=OP.subtract)

            # ---- combine sides + scale stats (big ops)
            nc.vector.tensor_tensor(out=stA_add[:], in0=stA_add[:],
                                    in1=stB_add[:], op=OP.add)
            nc.vector.tensor_tensor(out=stA_mn[:], in0=stA_mn[:],
                                    in1=stB_mn[:], op=OP.min)
            nc.vector.tensor_tensor(out=stA_mx[:], in0=stA_mx[:],
                                    in1=stB_mx[:], op=OP.max)
            nc.vector.tensor_tensor(out=stA_mn[:], in0=stA_mn[:],
                                    in1=dinvb[:], op=OP.mult)
            nc.vector.tensor_tensor(out=stA_mx[:], in0=stA_mx[:],
                                    in1=dinvb[:], op=OP.mult)

            # ---- E: combine matmuls (feature-major h out) + bias + relu
            for g in range(NPADC // 512):
                gsl = slice(g * 512, (g + 1) * 512)
                mean_g = rhp.tile([128, 512], bf16, tag="mean_g")
                add_g = rhp.tile([128, 512], bf16, tag="add_g")
                nc.vector.tensor_tensor(out=mean_g[:], in0=stA_add[:, gsl],
                                        in1=ddegb[:, gsl], op=OP.mult)
                nc.vector.tensor_tensor(out=add_g[:], in0=stA_add[:, gsl],
                                        in1=dinvb[:, gsl], op=OP.mult)
                psc = pc.tile([128, 512], f32, tag="ps_cmb")
                for k, st in enumerate((mean_g[:], add_g[:],
                                        stA_mn[:, gsl], stA_mx[:, gsl])):
                    nc.tensor.matmul(psc[:], lhsT=ct[:, k, :], rhs=st,
                                     start=(k == 0), stop=(k == 3))
                nc.scalar.activation(hT[:, gsl], psc[:], AF.Relu,
                                     bias=bt[:], scale=1.0)

        # ---- logits + log_softmax (constant-shift, batched 4 chunks)
        wout = sb.tile([128, NCLS], f32, tag="wout")
        bout4 = sb.tile([128, 4, NCLS], f32, tag="bout4")
        nc.sync.dma_start(wout[:], t_wout.ap())
        nc.sync.dma_start(bout4[:], t_bout4.ap())
        for q in range(NCH // 4):
            ps4 = plg.tile([128, 4, NCLS], f32, tag="ps_lg")
            for k in range(4):
                j = q * 4 + k
                nc.tensor.matmul(ps4[:, k, :],
                                 lhsT=hT[:, j * 128:(j + 1) * 128],
                                 rhs=wout[:], start=True, stop=True)
            lg4 = lhsp.tile([128, 4, NCLS], f32, tag="lg4")
            nc.vector.tensor_tensor(out=lg4[:], in0=ps4[:], in1=bout4[:],
                                    op=OP.add)
            ex4 = lhsp.tile([128, 4, NCLS], f32, tag="ex4")
            nc.scalar.activation(ex4[:], lg4[:], AF.Exp)
            se4 = lhsp.tile([128, 4], f32, tag="se4")
            nc.vector.tensor_reduce(out=se4[:], in_=ex4[:], axis=AX, op=OP.add)
            ls4 = lhsp.tile([128, 4], f32, tag="ls4")
            nc.scalar.activation(ls4[:], se4[:], AF.Ln)
            for k in range(4):
                nc.vector.tensor_scalar_sub(lg4[:, k, :], lg4[:, k, :],
                                            ls4[:, k:k + 1])
            nc.sync.dma_start(
                t_out.ap().rearrange("(a p) n -> p a n", p=128)[:, 4 * q:4 * q + 4, :],
                lg4[:])

    nc.compile()
    return nc


_CACHE = {}


def kernel(x, edge_index, W0, C0, b0, W1, C1, b1, Wout, bout,
           trace=False, _want_results=False):
    x = np.asarray(x, dtype=np.float32)
    per_core, meta = _host_prep(x, edge_index)
    key = (tuple(meta["SA"]), tuple(meta["SB"]))
    if key not in _CACHE:
        _CACHE[key] = _build_program(meta)
    nc = _CACHE[key]

    import ml_dtypes
    shared = dict(
        W0T=np.ascontiguousarray(np.asarray(W0, np.float32).T),
        W1T=np.ascontiguousarray(np.asarray(W1, np.float32).T),
        C0T=np.ascontiguousarray(np.asarray(C0, np.float32).T).reshape(4, 128, 128).astype(ml_dtypes.bfloat16),
        C1T=np.ascontiguousarray(np.asarray(C1, np.float32).T).reshape(4, 128, 128).astype(ml_dtypes.bfloat16),
        b0=np.asarray(b0, np.float32).reshape(128, 1),
        b1=np.asarray(b1, np.float32).reshape(128, 1),
        WoutT=np.ascontiguousarray(np.asarray(Wout, np.float32).T),
        bout4=np.broadcast_to(np.asarray(bout, np.float32), (128, 4, NCLS)).copy(),
    )
    in_maps = []
    for c in range(CORES):
        d = per_core[c]
        m = dict(shared)
        m.update(xT=d["xT"], dinv_scale=d["dinv_scale"],
                 dinvb=d["dinvb"].astype(ml_dtypes.bfloat16),
                 ddegb=d["ddegb"].astype(ml_dtypes.bfloat16),
                 npadbA=d["npadbA"].astype(ml_dtypes.bfloat16),
                 npadbB=d["npadbB"].astype(ml_dtypes.bfloat16),
                 eidxA=d["eidxA"], eidxB=d["eidxB"])
        in_maps.append(m)

    res = bass_utils.run_bass_kernel_spmd(
        nc, in_maps, core_ids=list(range(CORES)), trace=trace)

    out = np.zeros((N, NCLS), dtype=np.float32)
    for c in range(CORES):
        o = res.results[c]["out"]
        d = per_core[c]
        out[d["gl"][d["real"]]] = o[d["real"]]
    if _want_results:
        return out, res
    return out


# revision 10
# speedup vs baseline: 1.7130x; 1.0876x over previous
"""GCN (2-layer, mean/add/min/max aggregation) Trainium2 Bass kernel, 8 NeuronCores.

v3: table-free edge gather + fused pipeline. Nodes partitioned by destination
across 8 cores (5000/core, one degree-sorted phase of 40 x 128-lane blocks).
Per layer each core computes g = dinv * (h @ W.T) for its shard in both
node-major (bf16 -> gsh -> AllGather -> DRAM gfull, double-buffered per layer)
and feature-major (SBUF gT, used as the self-loop message). Non-self edge
messages are gathered feature-major straight from DRAM (dma_gather
transpose=True, no SBUF staging table). The int16 gather-index limit (<32768)
is handled with two overlapping source windows A=[0,32768) and B=[8192,40960):
each dest's edges split between two message buffers, balanced ~deg/2 per side
inside each 128-lane block to keep slot padding low. Per block both sides are
segment-reduced (add f32 with exact pad correction, min/max bf16). Per 512-lane
group, side combining + self fold (gT) + dinv scaling + the 512->128 combine
matmul (bf16) + bias/ReLU + the next layer's g matmuls (or the final logits
with constant-shift log_softmax) are emitted as soon as that group's chunks
land, so they hide under the Pool-engine descriptor generation that dominates
the kernel. The AllGather is split in two lane-piece collectives that fire
under the previous layer's gather tail (gfull is double-buffered to avoid the
WAR serialization).
"""
import sys

sys.path.insert(0, "/opt/trn_rl_repo")

import numpy as np
from contextlib import ExitStack

import concourse.bacc as bacc
import concourse.tile as tile
import concourse.mybir as mybir
from concourse import bass_utils

N = 40000
E = 640000
D = 128
NCLS = 40
CORES = 8
NPC = N // CORES            # 5000 nodes/core
NPADC = 5120                # padded nodes/core (40 blocks of 128 lanes)
BLK = NPADC // 128          # 40 blocks
NG = CORES * NPADC          # 40960 global g rows
WIN = 32768                 # int16 window size
BOFF = NG - WIN             # 8192: window B covers [8192, 40960)
PIECES = 2
PSZ = NPADC // PIECES       # 2560 lanes per AllGather piece
MSG_COLS = 6144
GRP = 512                   # lanes per fused combine/E/A group
NGRP = NPADC // GRP         # 10 groups


def _wrap_idx(idx):
    """int16 -> [128, n/16] wrapped (i -> [i%16, i//16]) and replicated x8."""
    idx = np.asarray(idx, dtype=np.int16)
    n = len(idx)
    assert n % 16 == 0
    cols = n // 16
    base = np.zeros((16, cols), dtype=np.int16)
    base[np.arange(n) % 16, np.arange(n) // 16] = idx
    return np.tile(base, (8, 1))


def _host_prep(x, edge_index):
    # deg/dinv include the appended self-loops (as in the reference)
    row = np.asarray(edge_index[0]).astype(np.int64)   # E original edges only
    col = np.asarray(edge_index[1]).astype(np.int64)
    deg = (np.bincount(col, minlength=N) + 1).astype(np.float64)
    dinv = deg ** -0.5
    ddeg = dinv / deg

    # per-core degree-sorted lane order; gpos = global row in gfull
    # (piece-major layout: (c, lane) -> (lane//PSZ)*8*PSZ + c*PSZ + lane%PSZ)
    lane_of_node = np.zeros(N, dtype=np.int64)
    node_of_lane = np.full((CORES, NPADC), -1, dtype=np.int64)
    for c in range(CORES):
        degs_c = deg[c * NPC:(c + 1) * NPC]
        o = np.argsort(-degs_c, kind="stable")
        lane_of_node[c * NPC + o] = np.arange(NPC)
        node_of_lane[c, :NPC] = c * NPC + o
    lane_all = lane_of_node.copy()
    core_all = np.repeat(np.arange(CORES), NPC)
    gpos = (lane_all // PSZ) * CORES * PSZ + core_all * PSZ + (lane_all % PSZ)

    # per-core non-self edge lists sorted by (lane, side-category)
    per_core_edges = []
    nA_all = np.zeros((CORES, NPADC), dtype=np.int64)
    nB_all = np.zeros((CORES, NPADC), dtype=np.int64)
    for c in range(CORES):
        sel = (col >= c * NPC) & (col < (c + 1) * NPC)
        lanes = lane_of_node[col[sel]]
        gp = gpos[row[sel]]
        cat = np.ones(len(gp), dtype=np.int64)          # free
        cat[gp < BOFF] = 0                              # must-A
        cat[gp >= WIN] = 2                              # must-B
        sidx = np.lexsort((cat, lanes))
        lanes, gp, cat = lanes[sidx], gp[sidx], cat[sidx]
        cnt = np.bincount(lanes, minlength=NPADC)
        off = np.zeros(NPADC + 1, dtype=np.int64)
        off[1:] = np.cumsum(cnt)
        mA = np.bincount(lanes[cat == 0], minlength=NPADC)
        mB = np.bincount(lanes[cat == 2], minlength=NPADC)
        t = (cnt + 1) // 2
        nA = np.minimum(np.maximum(t, mA), cnt - mB)
        nB = cnt - nA
        real = cnt > 0
        bad = real & ((nA == 0) | (nB == 0))
        assert not bad.any(), "dest with an unpopulatable gather side"
        per_core_edges.append((lanes, gp, off, cnt))
        nA_all[c], nB_all[c] = nA, nB

    # global uniform slot counts per (block, side)
    SA = np.maximum(nA_all.reshape(CORES, BLK, 128).max(axis=(0, 2)), 1)
    SB = np.maximum(nB_all.reshape(CORES, BLK, 128).max(axis=(0, 2)), 1)
    PA = np.zeros(BLK + 1, dtype=np.int64)
    PA[1:] = np.cumsum(128 * SA)
    PB = np.zeros(BLK + 1, dtype=np.int64)
    PB[1:] = np.cumsum(128 * SB)
    colsA, colsB = int(PA[-1]), int(PB[-1])

    per_core = []
    for c in range(CORES):
        lanes, gp, off, cnt = per_core_edges[c]
        nA, nB = nA_all[c], nB_all[c]
        blk = np.arange(NPADC) // 128
        lane_in_blk = np.arange(NPADC) % 128
        baseA = PA[blk] + lane_in_blk * SA[blk]
        baseB = PB[blk] + lane_in_blk * SB[blk]

        rank = np.arange(len(lanes)) - off[lanes]
        isA = rank < nA[lanes]
        posA = baseA[lanes] + rank
        posB = baseB[lanes] + (rank - nA[lanes])
        tokA_real = gp[isA]
        tokB_real = gp[~isA] - BOFF
        assert len(tokA_real) == 0 or (0 <= tokA_real.min() and tokA_real.max() < WIN)
        assert len(tokB_real) == 0 or (0 <= tokB_real.min() and tokB_real.max() < WIN)

        # slot-0 token per lane (pads duplicate it); 0 for empty lanes
        tok0A = np.zeros(NPADC, dtype=np.int64)
        tok0A[lanes[isA & (rank == 0)]] = gp[isA & (rank == 0)]
        tok0B = np.zeros(NPADC, dtype=np.int64)
        firstB = (~isA) & (rank == nA[lanes])
        tok0B[lanes[firstB]] = gp[firstB] - BOFF

        edA = np.zeros(colsA, dtype=np.int64)
        edB = np.zeros(colsB, dtype=np.int64)
        for b in range(BLK):
            lv = slice(b * 128, (b + 1) * 128)
            edA[PA[b]:PA[b + 1]] = np.repeat(tok0A[lv], SA[b])
            edB[PB[b]:PB[b + 1]] = np.repeat(tok0B[lv], SB[b])
        edA[posA[isA]] = tokA_real
        edB[posB[~isA]] = tokB_real

        npadA = (SA[blk] - nA).astype(np.float64)
        npadB = (SB[blk] - nB).astype(np.float64)

        nodes = node_of_lane[c]
        real = nodes >= 0
        gl = np.where(real, nodes, 0)
        xp = np.zeros((NPADC, D), dtype=np.float32)
        xp[real] = np.asarray(x)[gl[real]]
        xT = np.ascontiguousarray(xp.T)
        dinv_l = np.where(real, dinv[gl], 0.0)
        ddeg_l = np.where(real, ddeg[gl], 0.0)

        per_core.append(dict(
            xT=xT,
            dinv_scale=np.ascontiguousarray(
                dinv_l.reshape(BLK, 128).T).astype(np.float32),
            dinvb=np.broadcast_to(dinv_l, (128, NPADC)).astype(np.float32).copy(),
            ddegb=np.broadcast_to(ddeg_l, (128, NPADC)).astype(np.float32).copy(),
            npadbA=np.broadcast_to(npadA, (128, NPADC)).astype(np.float32).copy(),
            npadbB=np.broadcast_to(npadB, (128, NPADC)).astype(np.float32).copy(),
            eidxA=_wrap_idx(edA), eidxB=_wrap_idx(edB),
            real=real, gl=gl,
        ))
    meta = dict(SA=SA, SB=SB, PA=PA, PB=PB, colsA=colsA, colsB=colsB)
    return per_core, meta


def _chunks(S, P, max_cols):
    out, cur, cur_cols = [], [], 0
    for b in range(BLK):
        w = 128 * int(S[b])
        if cur and cur_cols + w > max_cols:
            out.append(cur)
            cur, cur_cols = [], 0
        cur.append(b)
        cur_cols += w
    if cur:
        out.append(cur)
    return out


def _build_program(meta):
    SA, SB, PA, PB = meta["SA"], meta["SB"], meta["PA"], meta["PB"]
    colsA, colsB = meta["colsA"], meta["colsB"]
    f32, bf16, i16 = mybir.dt.float32, mybir.dt.bfloat16, mybir.dt.int16
    AX = mybir.AxisListType.X
    OP = mybir.AluOpType
    AF = mybir.ActivationFunctionType

    nc = bacc.Bacc("TRN2", target_bir_lowering=False, debug=False,
                   num_devices=CORES)
    t_xT = nc.dram_tensor("xT", [128, NPADC], f32, kind="ExternalInput")
    t_w = [nc.dram_tensor(f"W{l}T", [128, 128], f32, kind="ExternalInput") for l in range(2)]
    t_c = [nc.dram_tensor(f"C{l}T", [4, 128, 128], bf16, kind="ExternalInput") for l in range(2)]
    t_b = [nc.dram_tensor(f"b{l}", [128, 1], f32, kind="ExternalInput") for l in range(2)]
    t_wout = nc.dram_tensor("WoutT", [128, NCLS], f32, kind="ExternalInput")
    t_bout4 = nc.dram_tensor("bout4", [128, 4, NCLS], f32, kind="ExternalInput")
    t_dsc = nc.dram_tensor("dinv_scale", [128, BLK], f32, kind="ExternalInput")
    t_dinvb = nc.dram_tensor("dinvb", [128, NPADC], bf16, kind="ExternalInput")
    t_ddegb = nc.dram_tensor("ddegb", [128, NPADC], bf16, kind="ExternalInput")
    t_npadA = nc.dram_tensor("npadbA", [128, NPADC], bf16, kind="ExternalInput")
    t_npadB = nc.dram_tensor("npadbB", [128, NPADC], bf16, kind="ExternalInput")
    t_eidxA = nc.dram_tensor("eidxA", [128, colsA // 16], i16, kind="ExternalInput")
    t_eidxB = nc.dram_tensor("eidxB", [128, colsB // 16], i16, kind="ExternalInput")
    t_out = nc.dram_tensor("out", [NPADC, NCLS], f32, kind="ExternalOutput")
    t_gsh = [nc.dram_tensor(f"gsh{p}", [PSZ, D], bf16, kind="Internal")
             for p in range(PIECES)]
    t_gfull = [nc.dram_tensor(f"gfull{l}", [NG, D], bf16, kind="Internal")
               for l in range(2)]

    chA = _chunks(SA, PA, MSG_COLS)
    chB = _chunks(SB, PB, MSG_COLS)
    # merged stream: interleave sides ordered by last covered block
    merged = sorted(
        [("A", ch) for ch in chA] + [("B", ch) for ch in chB],
        key=lambda sc: (sc[1][-1], sc[0]))

    with tile.TileContext(nc) as tc, ExitStack() as ctx:
        sb = ctx.enter_context(tc.tile_pool(name="sb", bufs=1))
        lhsp = ctx.enter_context(tc.tile_pool(name="lhsp", bufs=3))
        msgp = ctx.enter_context(tc.tile_pool(name="msgp", bufs=2))
        rhp = ctx.enter_context(tc.tile_pool(name="rhp", bufs=2))
        pg = ctx.enter_context(tc.tile_pool(name="pg", bufs=2, space="PSUM"))
        pc = ctx.enter_context(tc.tile_pool(name="pc", bufs=2, space="PSUM"))
        plg = ctx.enter_context(tc.tile_pool(name="plg", bufs=2, space="PSUM"))

        hT = sb.tile([128, NPADC], f32, tag="hT")
        gT = sb.tile([128, NPADC], bf16, tag="gT")
        dsc = sb.tile([128, BLK], f32, tag="dsc")
        dinvb = sb.tile([128, NPADC], bf16, tag="dinvb")
        ddegb = sb.tile([128, NPADC], bf16, tag="ddegb")
        npadA = sb.tile([128, NPADC], bf16, tag="npadA")
        npadB = sb.tile([128, NPADC], bf16, tag="npadB")
        eixA = sb.tile([128, colsA // 16], i16, tag="eixA")
        eixB = sb.tile([128, colsB // 16], i16, tag="eixB")
        wout = sb.tile([128, NCLS], f32, tag="wout")
        bout4 = sb.tile([128, 4, NCLS], f32, tag="bout4")
        wts, cts, bts = [], [], []
        for l in range(2):
            wts.append(sb.tile([128, 128], f32, tag=f"wt{l}", name=f"wt{l}"))
            cts.append(sb.tile([128, 4, 128], bf16, tag=f"ct{l}", name=f"ct{l}"))
            bts.append(sb.tile([128, 1], f32, tag=f"bt{l}", name=f"bt{l}"))
        nc.sync.dma_start(wts[0][:], t_w[0].ap())
        nc.sync.dma_start(dsc[:], t_dsc.ap())
        nc.sync.dma_start(eixA[:], t_eidxA.ap())
        nc.sync.dma_start(eixB[:], t_eidxB.ap())
        nc.sync.dma_start(dinvb[:], t_dinvb.ap())
        nc.sync.dma_start(ddegb[:], t_ddegb.ap())
        nc.sync.dma_start(npadA[:], t_npadA.ap())
        nc.sync.dma_start(npadB[:], t_npadB.ap())
        nc.sync.dma_start(wts[1][:], t_w[1].ap())
        for l in range(2):
            nc.sync.dma_start(cts[l][:], t_c[l].ap().rearrange("k p f -> p k f"))
            nc.sync.dma_start(bts[l][:], t_b[l].ap())
        nc.sync.dma_start(wout[:], t_wout.ap())
        nc.sync.dma_start(bout4[:], t_bout4.ap())

        stA_add = sb.tile([128, NPADC], f32, tag="stA_add")
        stB_add = sb.tile([128, NPADC], f32, tag="stB_add")
        stA_mn = sb.tile([128, NPADC], bf16, tag="stA_mn")
        stB_mn = sb.tile([128, NPADC], bf16, tag="stB_mn")
        stA_mx = sb.tile([128, NPADC], bf16, tag="stA_mx")
        stB_mx = sb.tile([128, NPADC], bf16, tag="stB_mx")

        def g_chunk(l, j):
            """node-major g for 128 lanes j*128.. -> gsh piece; lhs from hT/xT."""
            if l == 0:
                lhs = lhsp.tile([128, 128], f32, tag="lhs")
                nc.sync.dma_start(lhs[:], t_xT.ap()[:, j * 128:(j + 1) * 128])
                lhs_ap = lhs[:]
            else:
                lhs_ap = hT[:, j * 128:(j + 1) * 128]
            ps = pg.tile([128, 128], f32, tag="ps_g")
            nc.tensor.matmul(ps[:], lhsT=lhs_ap, rhs=wts[l][:], start=True, stop=True)
            gt = lhsp.tile([128, 128], bf16, tag="gt")
            nc.scalar.activation(gt[:], ps[:], AF.Copy, scale=dsc[:, j:j + 1])
            p = j // (PSZ // 128)
            jj = j - p * (PSZ // 128)
            nc.sync.dma_start(
                t_gsh[p].ap().rearrange("(a p) d -> p a d", p=128)[:, jj, :], gt[:])
            # feature-major gT (self message) via wide matmul would need 512
            # cols; do it per 128 via PE transpose-free path: gT = dinv * (W @ lhs)
            psT = pg.tile([128, 128], f32, tag="ps_gT")
            nc.tensor.matmul(psT[:], lhsT=wts[l][:], rhs=lhs_ap, start=True, stop=True)
            nc.vector.tensor_tensor(
                out=gT[:, j * 128:(j + 1) * 128], in0=psT[:],
                in1=dinvb[:, j * 128:(j + 1) * 128], op=OP.mult)

        def fused_group(l, g):
            """combine + scale + E-matmul for lanes [g*GRP,(g+1)*GRP); then
            next-layer g chunks (l==0) or logits (l==1)."""
            gsl = slice(g * GRP, (g + 1) * GRP)
            nc.vector.tensor_tensor(out=stA_add[:, gsl], in0=stA_add[:, gsl],
                                    in1=stB_add[:, gsl], op=OP.add)
            nc.vector.tensor_tensor(out=stA_mn[:, gsl], in0=stA_mn[:, gsl],
                                    in1=stB_mn[:, gsl], op=OP.min)
            nc.vector.tensor_tensor(out=stA_mx[:, gsl], in0=stA_mx[:, gsl],
                                    in1=stB_mx[:, gsl], op=OP.max)
            # fold in the self-loop message (gT)
            nc.vector.tensor_tensor(out=stA_add[:, gsl], in0=stA_add[:, gsl],
                                    in1=gT[:, gsl], op=OP.add)
            nc.vector.tensor_tensor(out=stA_mn[:, gsl], in0=stA_mn[:, gsl],
                                    in1=gT[:, gsl], op=OP.min)
            nc.vector.tensor_tensor(out=stA_mx[:, gsl], in0=stA_mx[:, gsl],
                                    in1=gT[:, gsl], op=OP.max)
            # scale: mean/add from f32 accumulator; mn/mx in place
            mean_g = rhp.tile([128, GRP], bf16, tag="mean_g")
            add_g = rhp.tile([128, GRP], bf16, tag="add_g")
            nc.vector.tensor_tensor(out=mean_g[:], in0=stA_add[:, gsl],
                                    in1=ddegb[:, gsl], op=OP.mult)
            nc.vector.tensor_tensor(out=add_g[:], in0=stA_add[:, gsl],
                                    in1=dinvb[:, gsl], op=OP.mult)
            nc.vector.tensor_tensor(out=stA_mn[:, gsl], in0=stA_mn[:, gsl],
                                    in1=dinvb[:, gsl], op=OP.mult)
            nc.vector.tensor_tensor(out=stA_mx[:, gsl], in0=stA_mx[:, gsl],
                                    in1=dinvb[:, gsl], op=OP.mult)
            psc = pc.tile([128, GRP], f32, tag="ps_cmb")
            for k, st in enumerate((mean_g[:], add_g[:],
                                    stA_mn[:, gsl], stA_mx[:, gsl])):
                nc.tensor.matmul(psc[:], lhsT=cts[l][:, k, :], rhs=st,
                                 start=(k == 0), stop=(k == 3))
            nc.scalar.activation(hT[:, gsl], psc[:], AF.Relu,
                                 bias=bts[l][:], scale=1.0)
            if l == 0:
                for j in range(g * 4, g * 4 + 4):
                    g_chunk(1, j)
            else:
                logits_group(g)

        def logits_group(q):
            ps4 = plg.tile([128, 4, NCLS], f32, tag="ps_lg")
            for k in range(4):
                j = q * 4 + k
                nc.tensor.matmul(ps4[:, k, :],
                                 lhsT=hT[:, j * 128:(j + 1) * 128],
                                 rhs=wout[:], start=True, stop=True)
            lg4 = lhsp.tile([128, 4, NCLS], f32, tag="lg4")
            nc.vector.tensor_tensor(out=lg4[:], in0=ps4[:], in1=bout4[:],
                                    op=OP.add)
            ex4 = lhsp.tile([128, 4, NCLS], f32, tag="ex4")
            nc.scalar.activation(ex4[:], lg4[:], AF.Exp)
            se4 = lhsp.tile([128, 4], f32, tag="se4")
            nc.vector.tensor_reduce(out=se4[:], in_=ex4[:], axis=AX, op=OP.add)
            ls4 = lhsp.tile([128, 4], f32, tag="ls4")
            nc.scalar.activation(ls4[:], se4[:], AF.Ln)
            for k in range(4):
                nc.vector.tensor_scalar_sub(lg4[:, k, :], lg4[:, k, :],
                                            ls4[:, k:k + 1])
            nc.sync.dma_start(
                t_out.ap().rearrange("(a p) n -> p a n", p=128)[:, 4 * q:4 * q + 4, :],
                lg4[:])

        def ag_piece(p, dst):
            nc.gpsimd.collective_compute(
                "AllGather", OP.bypass, replica_groups=[list(range(CORES))],
                ins=[t_gsh[p].ap()],
                outs=[t_gfull[dst].ap()[p * CORES * PSZ:(p + 1) * CORES * PSZ, :]])

        # ---- layer 0 A-stage from xT, AllGather pieces as they complete
        for p in range(PIECES):
            for j in range(p * (PSZ // 128), (p + 1) * (PSZ // 128)):
                g_chunk(0, j)
            ag_piece(p, 0)

        for l in range(2):
            # ---- gathers + reduces, fused groups as blocks complete
            covA = np.zeros(BLK, dtype=bool)
            covB = np.zeros(BLK, dtype=bool)
            next_g = 0
            ag0_emitted = False
            for side, ch in merged:
                S, P, eix, lo, hi = (
                    (SA, PA, eixA, 0, WIN) if side == "A"
                    else (SB, PB, eixB, BOFF, NG))
                st_add = stA_add if side == "A" else stB_add
                st_mn = stA_mn if side == "A" else stB_mn
                st_mx = stA_mx if side == "A" else stB_mx
                npadS = npadA if side == "A" else npadB
                q0 = int(P[ch[0]])
                qn = int(P[ch[-1] + 1]) - q0
                msg = msgp.tile([128, 1, MSG_COLS], bf16, tag="msg")
                nc.gpsimd.dma_gather(
                    out_ap=msg[:, :, :qn],
                    in_ap=t_gfull[l].ap()[lo:hi, :],
                    idxs_ap=eix[:, q0 // 16:(q0 + qn) // 16],
                    num_idxs=qn, num_idxs_reg=qn, elem_size=D,
                    transpose=True, single_packet=False)
                for b in ch:
                    sbl = int(S[b])
                    cb = int(P[b]) - q0
                    view = msg[:, 0, cb:cb + 128 * sbl].rearrange(
                        "p (l s) -> p l s", s=sbl)
                    dsl = slice(b * 128, (b + 1) * 128)
                    nc.vector.tensor_reduce(
                        out=st_add[:, dsl], in_=view, axis=AX, op=OP.add)
                    nc.vector.tensor_reduce(
                        out=st_mn[:, dsl], in_=view, axis=AX, op=OP.min)
                    nc.vector.tensor_reduce(
                        out=st_mx[:, dsl], in_=view, axis=AX, op=OP.max)
                    tmp = lhsp.tile([128, 128], f32, tag="tmp")
                    nc.vector.tensor_tensor(
                        out=tmp[:], in0=view[:, :, 0], in1=npadS[:, dsl],
                        op=OP.mult)
                    nc.vector.tensor_tensor(
                        out=st_add[:, dsl], in0=st_add[:, dsl],
                        in1=tmp[:], op=OP.subtract)
                    if side == "A":
                        covA[b] = True
                    else:
                        covB[b] = True
                while next_g < NGRP and covA[next_g * 4:(next_g + 1) * 4].all() \
                        and covB[next_g * 4:(next_g + 1) * 4].all():
                    fused_group(l, next_g)
                    next_g += 1
                # fire layer-1 AllGather piece 0 once its lanes (+1 group of
                # slack so the Pool never stalls on it) are through E/A
                if l == 0 and not ag0_emitted and next_g >= NGRP // 2 + 1:
                    ag_piece(0, 1)
                    ag0_emitted = True
            assert next_g == NGRP
            if l == 0:
                assert ag0_emitted
                ag_piece(1, 1)

    nc.compile()
    return nc


_CACHE = {}


def kernel(x, edge_index, W0, C0, b0, W1, C1, b1, Wout, bout,
           trace=False, _want_results=False):
    x = np.asarray(x, dtype=np.float32)
    per_core, meta = _host_prep(x, edge_index)
    key = (tuple(meta["SA"]), tuple(meta["SB"]))
    if key not in _CACHE:
        _CACHE[key] = _build_program(meta)
    nc = _CACHE[key]

    import ml_dtypes
    shared = dict(
        W0T=np.ascontiguousarray(np.asarray(W0, np.float32).T),
        W1T=np.ascontiguousarray(np.asarray(W1, np.float32).T),
        C0T=np.ascontiguousarray(np.asarray(C0, np.float32).T).reshape(4, 128, 128).astype(ml_dtypes.bfloat16),
        C1T=np.ascontiguousarray(np.asarray(C1, np.float32).T).reshape(4, 128, 128).astype(ml_dtypes.bfloat16),
        b0=np.asarray(b0, np.float32).reshape(128, 1),
        b1=np.asarray(b1, np.float32).reshape(128, 1),
        WoutT=np.ascontiguousarray(np.asarray(Wout, np.float32).T),
        bout4=np.broadcast_to(np.asarray(bout, np.float32), (128, 4, NCLS)).copy(),
    )
    in_maps = []
    for c in range(CORES):
        d = per_core[c]
        m = dict(shared)
        m.update(xT=d["xT"], dinv_scale=d["dinv_scale"],
                 dinvb=d["dinvb"].astype(ml_dtypes.bfloat16),
                 ddegb=d["ddegb"].astype(ml_dtypes.bfloat16),
                 npadbA=d["npadbA"].astype(ml_dtypes.bfloat16),
                 npadbB=d["npadbB"].astype(ml_dtypes.bfloat16),
                 eidxA=d["eidxA"], eidxB=d["eidxB"])
        in_maps.append(m)

    res = bass_utils.run_bass_kernel_spmd(
        nc, in_maps, core_ids=list(range(CORES)), trace=trace)

    out = np.zeros((N, NCLS), dtype=np.float32)
    for c in range(CORES):
        o = res.results[c]["out"]
        d = per_core[c]
        out[d["gl"][d["real"]]] = o[d["real"]]
    if _want_results:
        return out, res
    return out


# revision 12
# speedup vs baseline: 1.7208x; 1.0045x over previous
"""GCN (2-layer, mean/add/min/max aggregation) Trainium2 Bass kernel, 8 NeuronCores.

v3: table-free edge gather + fused pipeline. Nodes partitioned by destination
across 8 cores (5000/core, one degree-sorted phase of 40 x 128-lane blocks).
Per layer each core computes g = dinv * (h @ W.T) for its shard in both
node-major (bf16 -> gsh -> AllGather -> DRAM gfull, double-buffered per layer)
and feature-major (SBUF gT, used as the self-loop message). Non-self edge
messages are gathered feature-major straight from DRAM (dma_gather
transpose=True, no SBUF staging table). The int16 gather-index limit (<32768)
is handled with two overlapping source windows A=[0,32768) and B=[8192,40960):
each dest's edges split between two message buffers, balanced ~deg/2 per side
inside each 128-lane block to keep slot padding low. Per block both sides are
segment-reduced (add f32 with exact pad correction, min/max bf16). Per 512-lane
group, side combining + self fold (gT) + dinv scaling + the 512->128 combine
matmul (bf16) + bias/ReLU + the next layer's g matmuls (or the final logits
with constant-shift log_softmax) are emitted as soon as that group's chunks
land, so they hide under the Pool-engine descriptor generation that dominates
the kernel. The AllGather is split in two lane-piece collectives that fire
under the previous layer's gather tail (gfull is double-buffered to avoid the
WAR serialization).
"""
import sys

sys.path.insert(0, "/opt/trn_rl_repo")

import numpy as np
from contextlib import ExitStack

import concourse.bacc as bacc
import concourse.tile as tile
import concourse.mybir as mybir
from concourse import bass_utils

N = 40000
E = 640000
D = 128
NCLS = 40
CORES = 8
NPC = N // CORES            # 5000 nodes/core
NPADC = 5120                # padded nodes/core (40 blocks of 128 lanes)
BLK = NPADC // 128          # 40 blocks
NG = CORES * NPADC          # 40960 global g rows
WIN = 32768                 # int16 window size
BOFF = NG - WIN             # 8192: window B covers [8192, 40960)
PIECES = 2
PSZ = NPADC // PIECES       # 2560 lanes per AllGather piece
MSG_COLS = 6144
GRP = 512                   # lanes per fused combine/E/A group
NGRP = NPADC // GRP         # 10 groups


def _wrap_idx(idx):
    """int16 -> [128, n/16] wrapped (i -> [i%16, i//16]) and replicated x8."""
    idx = np.asarray(idx, dtype=np.int16)
    n = len(idx)
    assert n % 16 == 0
    cols = n // 16
    base = np.zeros((16, cols), dtype=np.int16)
    base[np.arange(n) % 16, np.arange(n) // 16] = idx
    return np.tile(base, (8, 1))


def _host_prep(x, edge_index):
    # deg/dinv include the appended self-loops (as in the reference)
    row = np.asarray(edge_index[0]).astype(np.int64)   # E original edges only
    col = np.asarray(edge_index[1]).astype(np.int64)
    deg = (np.bincount(col, minlength=N) + 1).astype(np.float64)
    dinv = deg ** -0.5
    ddeg = dinv / deg

    # per-core degree-sorted lane order; gpos = global row in gfull
    # (piece-major layout: (c, lane) -> (lane//PSZ)*8*PSZ + c*PSZ + lane%PSZ)
    lane_of_node = np.zeros(N, dtype=np.int64)
    node_of_lane = np.full((CORES, NPADC), -1, dtype=np.int64)
    for c in range(CORES):
        degs_c = deg[c * NPC:(c + 1) * NPC]
        o = np.argsort(-degs_c, kind="stable")
        lane_of_node[c * NPC + o] = np.arange(NPC)
        node_of_lane[c, :NPC] = c * NPC + o
    lane_all = lane_of_node.copy()
    core_all = np.repeat(np.arange(CORES), NPC)
    gpos = (lane_all // PSZ) * CORES * PSZ + core_all * PSZ + (lane_all % PSZ)

    # per-core non-self edge lists sorted by (lane, side-category)
    per_core_edges = []
    mA_all = np.zeros((CORES, NPADC), dtype=np.int64)
    mB_all = np.zeros((CORES, NPADC), dtype=np.int64)
    cnt_all = np.zeros((CORES, NPADC), dtype=np.int64)
    for c in range(CORES):
        sel = (col >= c * NPC) & (col < (c + 1) * NPC)
        lanes = lane_of_node[col[sel]]
        gp = gpos[row[sel]]
        cat = np.ones(len(gp), dtype=np.int64)          # free
        cat[gp < BOFF] = 0                              # must-A
        cat[gp >= WIN] = 2                              # must-B
        sidx = np.lexsort((cat, lanes))
        lanes, gp, cat = lanes[sidx], gp[sidx], cat[sidx]
        cnt = np.bincount(lanes, minlength=NPADC)
        off = np.zeros(NPADC + 1, dtype=np.int64)
        off[1:] = np.cumsum(cnt)
        mA_all[c] = np.bincount(lanes[cat == 0], minlength=NPADC)
        mB_all[c] = np.bincount(lanes[cat == 2], minlength=NPADC)
        cnt_all[c] = cnt
        per_core_edges.append((lanes, gp, off, cnt))

    # joint per-block side capacities: S_A + S_B ~ max block degree, with the
    # per-lane must counts respected; the window overlap absorbs the rest
    D_b = cnt_all.reshape(CORES, BLK, 128).max(axis=(0, 2))
    MA_b = mA_all.reshape(CORES, BLK, 128).max(axis=(0, 2))
    MB_b = mB_all.reshape(CORES, BLK, 128).max(axis=(0, 2))
    SA = np.maximum(np.maximum((D_b + 1) // 2, MA_b), 1)
    SB = np.maximum(np.maximum(D_b - SA, MB_b), 1)
    blk_of_lane = np.arange(NPADC) // 128
    nA_all = np.zeros((CORES, NPADC), dtype=np.int64)
    nB_all = np.zeros((CORES, NPADC), dtype=np.int64)
    for c in range(CORES):
        cnt, mA, mB = cnt_all[c], mA_all[c], mB_all[c]
        lo = np.maximum(mA, cnt - SB[blk_of_lane])
        hi = np.minimum(SA[blk_of_lane], cnt - mB)
        assert (lo <= hi).all()
        nA = np.clip((cnt + 1) // 2, lo, hi)
        nB = cnt - nA
        real = cnt > 0
        bad = real & ((nA == 0) | (nB == 0))
        assert not bad.any(), "dest with an unpopulatable gather side"
        nA_all[c], nB_all[c] = nA, nB
    PA = np.zeros(BLK + 1, dtype=np.int64)
    PA[1:] = np.cumsum(128 * SA)
    PB = np.zeros(BLK + 1, dtype=np.int64)
    PB[1:] = np.cumsum(128 * SB)
    colsA, colsB = int(PA[-1]), int(PB[-1])

    per_core = []
    for c in range(CORES):
        lanes, gp, off, cnt = per_core_edges[c]
        nA, nB = nA_all[c], nB_all[c]
        blk = np.arange(NPADC) // 128
        lane_in_blk = np.arange(NPADC) % 128
        baseA = PA[blk] + lane_in_blk * SA[blk]
        baseB = PB[blk] + lane_in_blk * SB[blk]

        rank = np.arange(len(lanes)) - off[lanes]
        isA = rank < nA[lanes]
        posA = baseA[lanes] + rank
        posB = baseB[lanes] + (rank - nA[lanes])
        tokA_real = gp[isA]
        tokB_real = gp[~isA] - BOFF
        assert len(tokA_real) == 0 or (0 <= tokA_real.min() and tokA_real.max() < WIN)
        assert len(tokB_real) == 0 or (0 <= tokB_real.min() and tokB_real.max() < WIN)

        # slot-0 token per lane (pads duplicate it); 0 for empty lanes
        tok0A = np.zeros(NPADC, dtype=np.int64)
        tok0A[lanes[isA & (rank == 0)]] = gp[isA & (rank == 0)]
        tok0B = np.zeros(NPADC, dtype=np.int64)
        firstB = (~isA) & (rank == nA[lanes])
        tok0B[lanes[firstB]] = gp[firstB] - BOFF

        edA = np.zeros(colsA, dtype=np.int64)
        edB = np.zeros(colsB, dtype=np.int64)
        for b in range(BLK):
            lv = slice(b * 128, (b + 1) * 128)
            edA[PA[b]:PA[b + 1]] = np.repeat(tok0A[lv], SA[b])
            edB[PB[b]:PB[b + 1]] = np.repeat(tok0B[lv], SB[b])
        edA[posA[isA]] = tokA_real
        edB[posB[~isA]] = tokB_real

        npadA = (SA[blk] - nA).astype(np.float64)
        npadB = (SB[blk] - nB).astype(np.float64)

        nodes = node_of_lane[c]
        real = nodes >= 0
        gl = np.where(real, nodes, 0)
        xp = np.zeros((NPADC, D), dtype=np.float32)
        xp[real] = np.asarray(x)[gl[real]]
        xT = np.ascontiguousarray(xp.T)
        dinv_l = np.where(real, dinv[gl], 0.0)
        ddeg_l = np.where(real, ddeg[gl], 0.0)

        per_core.append(dict(
            xT=xT,
            dinv_scale=np.ascontiguousarray(
                dinv_l.reshape(BLK, 128).T).astype(np.float32),
            dinvb=np.broadcast_to(dinv_l, (128, NPADC)).astype(np.float32).copy(),
            ddegb=np.broadcast_to(ddeg_l, (128, NPADC)).astype(np.float32).copy(),
            npadbA=np.broadcast_to(npadA, (128, NPADC)).astype(np.float32).copy(),
            npadbB=np.broadcast_to(npadB, (128, NPADC)).astype(np.float32).copy(),
            eidxA=_wrap_idx(edA), eidxB=_wrap_idx(edB),
            real=real, gl=gl,
        ))
    meta = dict(SA=SA, SB=SB, PA=PA, PB=PB, colsA=colsA, colsB=colsB)
    return per_core, meta


def _chunks(S, P, max_cols):
    out, cur, cur_cols = [], [], 0
    for b in range(BLK):
        w = 128 * int(S[b])
        if cur and cur_cols + w > max_cols:
            out.append(cur)
            cur, cur_cols = [], 0
        cur.append(b)
        cur_cols += w
    if cur:
        out.append(cur)
    return out


def _build_program(meta):
    SA, SB, PA, PB = meta["SA"], meta["SB"], meta["PA"], meta["PB"]
    colsA, colsB = meta["colsA"], meta["colsB"]
    f32, bf16, i16 = mybir.dt.float32, mybir.dt.bfloat16, mybir.dt.int16
    AX = mybir.AxisListType.X
    OP = mybir.AluOpType
    AF = mybir.ActivationFunctionType

    nc = bacc.Bacc("TRN2", target_bir_lowering=False, debug=False,
                   num_devices=CORES)
    t_xT = nc.dram_tensor("xT", [128, NPADC], f32, kind="ExternalInput")
    t_w = [nc.dram_tensor(f"W{l}T", [128, 128], f32 if l == 0 else bf16,
                         kind="ExternalInput") for l in range(2)]
    t_c = [nc.dram_tensor(f"C{l}T", [4, 128, 128], bf16, kind="ExternalInput") for l in range(2)]
    t_b = [nc.dram_tensor(f"b{l}", [128, 1], f32, kind="ExternalInput") for l in range(2)]
    t_wout = nc.dram_tensor("WoutT", [128, NCLS], bf16, kind="ExternalInput")
    t_bout4 = nc.dram_tensor("bout4", [128, 4, NCLS], f32, kind="ExternalInput")
    t_dsc = nc.dram_tensor("dinv_scale", [128, BLK], f32, kind="ExternalInput")
    t_dinvb = nc.dram_tensor("dinvb", [128, NPADC], bf16, kind="ExternalInput")
    t_ddegb = nc.dram_tensor("ddegb", [128, NPADC], bf16, kind="ExternalInput")
    t_npadA = nc.dram_tensor("npadbA", [128, NPADC], bf16, kind="ExternalInput")
    t_npadB = nc.dram_tensor("npadbB", [128, NPADC], bf16, kind="ExternalInput")
    t_eidxA = nc.dram_tensor("eidxA", [128, colsA // 16], i16, kind="ExternalInput")
    t_eidxB = nc.dram_tensor("eidxB", [128, colsB // 16], i16, kind="ExternalInput")
    t_out = nc.dram_tensor("out", [NPADC, NCLS], f32, kind="ExternalOutput")
    t_gsh = [nc.dram_tensor(f"gsh{p}", [PSZ, D], bf16, kind="Internal")
             for p in range(PIECES)]
    t_gfull = [nc.dram_tensor(f"gfull{l}", [NG, D], bf16, kind="Internal")
               for l in range(2)]

    chA = _chunks(SA, PA, MSG_COLS)
    chB = _chunks(SB, PB, MSG_COLS)
    # merged stream: interleave sides ordered by last covered block
    merged = sorted(
        [("A", ch) for ch in chA] + [("B", ch) for ch in chB],
        key=lambda sc: (sc[1][-1], sc[0]))

    with tile.TileContext(nc) as tc, ExitStack() as ctx:
        sb = ctx.enter_context(tc.tile_pool(name="sb", bufs=1))
        lhsp = ctx.enter_context(tc.tile_pool(name="lhsp", bufs=3))
        msgp = ctx.enter_context(tc.tile_pool(name="msgp", bufs=3))
        rhp = ctx.enter_context(tc.tile_pool(name="rhp", bufs=2))
        pg = ctx.enter_context(tc.tile_pool(name="pg", bufs=2, space="PSUM"))
        pc = ctx.enter_context(tc.tile_pool(name="pc", bufs=2, space="PSUM"))
        plg = ctx.enter_context(tc.tile_pool(name="plg", bufs=2, space="PSUM"))

        hT = sb.tile([128, NPADC], bf16, tag="hT")
        gT = sb.tile([128, NPADC], bf16, tag="gT")
        dsc = sb.tile([128, BLK], f32, tag="dsc")
        dinvb = sb.tile([128, NPADC], bf16, tag="dinvb")
        ddegb = sb.tile([128, NPADC], bf16, tag="ddegb")
        npadA = sb.tile([128, NPADC], bf16, tag="npadA")
        npadB = sb.tile([128, NPADC], bf16, tag="npadB")
        eixA = sb.tile([128, colsA // 16], i16, tag="eixA")
        eixB = sb.tile([128, colsB // 16], i16, tag="eixB")
        wout = sb.tile([128, NCLS], bf16, tag="wout")
        bout4 = sb.tile([128, 4, NCLS], f32, tag="bout4")
        wts, cts, bts = [], [], []
        for l in range(2):
            wts.append(sb.tile([128, 128], f32 if l == 0 else bf16,
                                tag=f"wt{l}", name=f"wt{l}"))
            cts.append(sb.tile([128, 4, 128], bf16, tag=f"ct{l}", name=f"ct{l}"))
            bts.append(sb.tile([128, 1], f32, tag=f"bt{l}", name=f"bt{l}"))
        nc.sync.dma_start(wts[0][:], t_w[0].ap())
        nc.sync.dma_start(dsc[:], t_dsc.ap())
        nc.sync.dma_start(eixA[:], t_eidxA.ap())
        nc.sync.dma_start(eixB[:], t_eidxB.ap())
        nc.sync.dma_start(dinvb[:], t_dinvb.ap())
        nc.sync.dma_start(ddegb[:], t_ddegb.ap())
        nc.sync.dma_start(npadA[:], t_npadA.ap())
        nc.sync.dma_start(npadB[:], t_npadB.ap())
        nc.sync.dma_start(wts[1][:], t_w[1].ap())
        for l in range(2):
            nc.sync.dma_start(cts[l][:], t_c[l].ap().rearrange("k p f -> p k f"))
            nc.sync.dma_start(bts[l][:], t_b[l].ap())
        nc.sync.dma_start(wout[:], t_wout.ap())
        nc.sync.dma_start(bout4[:], t_bout4.ap())

        stA_add = sb.tile([128, NPADC], f32, tag="stA_add")
        stB_add = sb.tile([128, NPADC], f32, tag="stB_add")
        stA_mn = sb.tile([128, NPADC], bf16, tag="stA_mn")
        stB_mn = sb.tile([128, NPADC], bf16, tag="stB_mn")
        stA_mx = sb.tile([128, NPADC], bf16, tag="stA_mx")
        stB_mx = sb.tile([128, NPADC], bf16, tag="stB_mx")

        def g_chunk(l, j):
            """node-major g for 128 lanes j*128.. -> gsh piece; lhs from hT/xT."""
            if l == 0:
                lhs = lhsp.tile([128, 128], f32, tag="lhs")
                nc.sync.dma_start(lhs[:], t_xT.ap()[:, j * 128:(j + 1) * 128])
                lhs_ap = lhs[:]
            else:
                lhs_ap = hT[:, j * 128:(j + 1) * 128]
            ps = pg.tile([128, 128], f32, tag="ps_g")
            nc.tensor.matmul(ps[:], lhsT=lhs_ap, rhs=wts[l][:], start=True, stop=True)
            gt = lhsp.tile([128, 128], bf16, tag="gt")
            nc.scalar.activation(gt[:], ps[:], AF.Copy, scale=dsc[:, j:j + 1])
            p = j // (PSZ // 128)
            jj = j - p * (PSZ // 128)
            nc.sync.dma_start(
                t_gsh[p].ap().rearrange("(a p) d -> p a d", p=128)[:, jj, :], gt[:])
            # feature-major gT (self message) via wide matmul would need 512
            # cols; do it per 128 via PE transpose-free path: gT = dinv * (W @ lhs)
            psT = pg.tile([128, 128], f32, tag="ps_gT")
            nc.tensor.matmul(psT[:], lhsT=wts[l][:], rhs=lhs_ap, start=True, stop=True)
            nc.vector.tensor_tensor(
                out=gT[:, j * 128:(j + 1) * 128], in0=psT[:],
                in1=dinvb[:, j * 128:(j + 1) * 128], op=OP.mult)

        def fused_group(l, g):
            """combine + scale + E-matmul for lanes [g*GRP,(g+1)*GRP); then
            next-layer g chunks (l==0) or logits (l==1)."""
            gsl = slice(g * GRP, (g + 1) * GRP)
            nc.vector.tensor_tensor(out=stA_add[:, gsl], in0=stA_add[:, gsl],
                                    in1=stB_add[:, gsl], op=OP.add)
            nc.vector.tensor_tensor(out=stA_mn[:, gsl], in0=stA_mn[:, gsl],
                                    in1=stB_mn[:, gsl], op=OP.min)
            nc.vector.tensor_tensor(out=stA_mx[:, gsl], in0=stA_mx[:, gsl],
                                    in1=stB_mx[:, gsl], op=OP.max)
            # fold in the self-loop message (gT)
            nc.vector.tensor_tensor(out=stA_add[:, gsl], in0=stA_add[:, gsl],
                                    in1=gT[:, gsl], op=OP.add)
            nc.vector.tensor_tensor(out=stA_mn[:, gsl], in0=stA_mn[:, gsl],
                                    in1=gT[:, gsl], op=OP.min)
            nc.vector.tensor_tensor(out=stA_mx[:, gsl], in0=stA_mx[:, gsl],
                                    in1=gT[:, gsl], op=OP.max)
            # scale: mean/add from f32 accumulator; mn/mx in place
            mean_g = rhp.tile([128, GRP], bf16, tag="mean_g")
            add_g = rhp.tile([128, GRP], bf16, tag="add_g")
            nc.vector.tensor_tensor(out=mean_g[:], in0=stA_add[:, gsl],
                                    in1=ddegb[:, gsl], op=OP.mult)
            nc.vector.tensor_tensor(out=add_g[:], in0=stA_add[:, gsl],
                                    in1=dinvb[:, gsl], op=OP.mult)
            nc.vector.tensor_tensor(out=stA_mn[:, gsl], in0=stA_mn[:, gsl],
                                    in1=dinvb[:, gsl], op=OP.mult)
            nc.vector.tensor_tensor(out=stA_mx[:, gsl], in0=stA_mx[:, gsl],
                                    in1=dinvb[:, gsl], op=OP.mult)
            psc = pc.tile([128, GRP], f32, tag="ps_cmb")
            for k, st in enumerate((mean_g[:], add_g[:],
                                    stA_mn[:, gsl], stA_mx[:, gsl])):
                nc.tensor.matmul(psc[:], lhsT=cts[l][:, k, :], rhs=st,
                                 start=(k == 0), stop=(k == 3))
            nc.scalar.activation(hT[:, gsl], psc[:], AF.Relu,
                                 bias=bts[l][:], scale=1.0)
            if l == 0:
                for j in range(g * 4, g * 4 + 4):
                    g_chunk(1, j)
            else:
                logits_group(g)

        def logits_group(q):
            ps4 = plg.tile([128, 4, NCLS], f32, tag="ps_lg")
            for k in range(4):
                j = q * 4 + k
                nc.tensor.matmul(ps4[:, k, :],
                                 lhsT=hT[:, j * 128:(j + 1) * 128],
                                 rhs=wout[:], start=True, stop=True)
            lg4 = lhsp.tile([128, 4, NCLS], f32, tag="lg4")
            nc.vector.tensor_tensor(out=lg4[:], in0=ps4[:], in1=bout4[:],
                                    op=OP.add)
            ex4 = lhsp.tile([128, 4, NCLS], f32, tag="ex4")
            nc.scalar.activation(ex4[:], lg4[:], AF.Exp)
            se4 = lhsp.tile([128, 4], f32, tag="se4")
            nc.vector.tensor_reduce(out=se4[:], in_=ex4[:], axis=AX, op=OP.add)
            ls4 = lhsp.tile([128, 4], f32, tag="ls4")
            nc.scalar.activation(ls4[:], se4[:], AF.Ln)
            for k in range(4):
                nc.vector.tensor_scalar_sub(lg4[:, k, :], lg4[:, k, :],
                                            ls4[:, k:k + 1])
            nc.sync.dma_start(
                t_out.ap().rearrange("(a p) n -> p a n", p=128)[:, 4 * q:4 * q + 4, :],
                lg4[:])

        def ag_piece(p, dst):
            nc.gpsimd.collective_compute(
                "AllGather", OP.bypass, replica_groups=[list(range(CORES))],
                ins=[t_gsh[p].ap()],
                outs=[t_gfull[dst].ap()[p * CORES * PSZ:(p + 1) * CORES * PSZ, :]])

        # ---- layer 0 A-stage from xT, AllGather pieces as they complete
        for p in range(PIECES):
            for j in range(p * (PSZ // 128), (p + 1) * (PSZ // 128)):
                g_chunk(0, j)
            ag_piece(p, 0)

        for l in range(2):
            # ---- gathers + reduces, fused groups as blocks complete
            covA = np.zeros(BLK, dtype=bool)
            covB = np.zeros(BLK, dtype=bool)
            next_g = 0
            ag0_emitted = False
            for side, ch in merged:
                S, P, eix, lo, hi = (
                    (SA, PA, eixA, 0, WIN) if side == "A"
                    else (SB, PB, eixB, BOFF, NG))
                st_add = stA_add if side == "A" else stB_add
                st_mn = stA_mn if side == "A" else stB_mn
                st_mx = stA_mx if side == "A" else stB_mx
                npadS = npadA if side == "A" else npadB
                q0 = int(P[ch[0]])
                qn = int(P[ch[-1] + 1]) - q0
                msg = msgp.tile([128, 1, MSG_COLS], bf16, tag="msg")
                nc.gpsimd.dma_gather(
                    out_ap=msg[:, :, :qn],
                    in_ap=t_gfull[l].ap()[lo:hi, :],
                    idxs_ap=eix[:, q0 // 16:(q0 + qn) // 16],
                    num_idxs=qn, num_idxs_reg=qn, elem_size=D,
                    transpose=True, single_packet=False)
                for b in ch:
                    sbl = int(S[b])
                    cb = int(P[b]) - q0
                    view = msg[:, 0, cb:cb + 128 * sbl].rearrange(
                        "p (l s) -> p l s", s=sbl)
                    dsl = slice(b * 128, (b + 1) * 128)
                    nc.vector.tensor_reduce(
                        out=st_add[:, dsl], in_=view, axis=AX, op=OP.add)
                    nc.vector.tensor_reduce(
                        out=st_mn[:, dsl], in_=view, axis=AX, op=OP.min)
                    nc.vector.tensor_reduce(
                        out=st_mx[:, dsl], in_=view, axis=AX, op=OP.max)
                    tmp = lhsp.tile([128, 128], f32, tag="tmp")
                    nc.vector.tensor_tensor(
                        out=tmp[:], in0=view[:, :, 0], in1=npadS[:, dsl],
                        op=OP.mult)
                    nc.vector.tensor_tensor(
                        out=st_add[:, dsl], in0=st_add[:, dsl],
                        in1=tmp[:], op=OP.subtract)
                    if side == "A":
                        covA[b] = True
                    else:
                        covB[b] = True
                while next_g < NGRP and covA[next_g * 4:(next_g + 1) * 4].all() \
                        and covB[next_g * 4:(next_g + 1) * 4].all():
                    fused_group(l, next_g)
                    next_g += 1
                # fire layer-1 AllGather piece 0 once its lanes (+1 group of
                # slack so the Pool never stalls on it) are through E/A
                if l == 0 and not ag0_emitted and next_g >= NGRP // 2 + 1:
                    ag_piece(0, 1)
                    ag0_emitted = True
            assert next_g == NGRP
            if l == 0:
                assert ag0_emitted
                ag_piece(1, 1)

    nc.compile()
    return nc


_CACHE = {}


def kernel(x, edge_index, W0, C0, b0, W1, C1, b1, Wout, bout,
           trace=False, _want_results=False):
    x = np.asarray(x, dtype=np.float32)
    per_core, meta = _host_prep(x, edge_index)
    key = (tuple(meta["SA"]), tuple(meta["SB"]))
    if key not in _CACHE:
        _CACHE[key] = _build_program(meta)
    nc = _CACHE[key]

    import ml_dtypes
    shared = dict(
        W0T=np.ascontiguousarray(np.asarray(W0, np.float32).T),
        W1T=np.ascontiguousarray(np.asarray(W1, np.float32).T).astype(ml_dtypes.bfloat16),
        C0T=np.ascontiguousarray(np.asarray(C0, np.float32).T).reshape(4, 128, 128).astype(ml_dtypes.bfloat16),
        C1T=np.ascontiguousarray(np.asarray(C1, np.float32).T).reshape(4, 128, 128).astype(ml_dtypes.bfloat16),
        b0=np.asarray(b0, np.float32).reshape(128, 1),
        b1=np.asarray(b1, np.float32).reshape(128, 1),
        WoutT=np.ascontiguousarray(np.asarray(Wout, np.float32).T).astype(ml_dtypes.bfloat16),
        bout4=np.broadcast_to(np.asarray(bout, np.float32), (128, 4, NCLS)).copy(),
    )
    in_maps = []
    for c in range(CORES):
        d = per_core[c]
        m = dict(shared)
        m.update(xT=d["xT"], dinv_scale=d["dinv_scale"],
                 dinvb=d["dinvb"].astype(ml_dtypes.bfloat16),
                 ddegb=d["ddegb"].astype(ml_dtypes.bfloat16),
                 npadbA=d["npadbA"].astype(ml_dtypes.bfloat16),
                 npadbB=d["npadbB"].astype(ml_dtypes.bfloat16),
                 eidxA=d["eidxA"], eidxB=d["eidxB"])
        in_maps.append(m)

    res = bass_utils.run_bass_kernel_spmd(
        nc, in_maps, core_ids=list(range(CORES)), trace=trace)

    out = np.zeros((N, NCLS), dtype=np.float32)
    for c in range(CORES):
        o = res.results[c]["out"]
        d = per_core[c]
        out[d["gl"][d["real"]]] = o[d["real"]]
    if _want_results:
        return out, res
    return out


# revision 14
# speedup vs baseline: 1.8622x; 1.0822x over previous
"""GCN (2-layer, mean/add/min/max aggregation) Trainium2 Bass kernel, 8 NeuronCores.

v3: table-free edge gather + fused pipeline. Nodes partitioned by destination
across 8 cores (5000/core, one degree-sorted phase of 40 x 128-lane blocks).
Per layer each core computes g = dinv * (h @ W.T) for its shard in both
node-major (bf16 -> gsh -> AllGather -> DRAM gfull, double-buffered per layer)
and feature-major (SBUF gT, used as the self-loop message). Non-self edge
messages are gathered feature-major straight from DRAM (dma_gather
transpose=True, no SBUF staging table). The int16 gather-index limit (<32768)
is handled with two overlapping source windows A=[0,32768) and B=[8192,40960):
each dest's edges split between two message buffers, balanced ~deg/2 per side
inside each 128-lane block to keep slot padding low. Per block both sides are
segment-reduced (add f32 with exact pad correction, min/max bf16). Per 512-lane
group, side combining + self fold (gT) + dinv scaling + the 512->128 combine
matmul (bf16) + bias/ReLU + the next layer's g matmuls (or the final logits
with constant-shift log_softmax) are emitted as soon as that group's chunks
land, so they hide under the Pool-engine descriptor generation that dominates
the kernel. The AllGather is split in two lane-piece collectives that fire
under the previous layer's gather tail (gfull is double-buffered to avoid the
WAR serialization).
"""
import sys

sys.path.insert(0, "/opt/trn_rl_repo")

import numpy as np
from contextlib import ExitStack

import concourse.bacc as bacc
import concourse.tile as tile
import concourse.mybir as mybir
from concourse import bass_utils

N = 40000
E = 640000
D = 128
NCLS = 40
CORES = 8
NPC = N // CORES            # 5000 nodes/core
NPADC = 5120                # padded nodes/core (40 blocks of 128 lanes)
BLK = NPADC // 128          # 40 blocks
NG = CORES * NPADC          # 40960 global g rows
WIN = 32768                 # int16 window size
BOFF = NG - WIN             # 8192: window B covers [8192, 40960)
PIECES = 5
PSZ = NPADC // PIECES       # 1024 lanes per AllGather piece
MSG_COLS = 6144
GRP = 512                   # lanes per fused combine/E/A group
NGRP = NPADC // GRP         # 10 groups


def _wrap_idx(idx):
    """int16 -> [128, n/16] wrapped (i -> [i%16, i//16]) and replicated x8."""
    idx = np.asarray(idx, dtype=np.int16)
    n = len(idx)
    assert n % 16 == 0
    cols = n // 16
    base = np.zeros((16, cols), dtype=np.int16)
    base[np.arange(n) % 16, np.arange(n) // 16] = idx
    return np.tile(base, (8, 1))


def _host_prep(x, edge_index):
    # deg/dinv include the appended self-loops (as in the reference)
    row = np.asarray(edge_index[0]).astype(np.int64)   # E original edges only
    col = np.asarray(edge_index[1]).astype(np.int64)
    deg = (np.bincount(col, minlength=N) + 1).astype(np.float64)
    dinv = deg ** -0.5
    ddeg = dinv / deg

    # per-core degree-sorted lane order; gpos = global row in gfull
    # (piece-major layout: (c, lane) -> (lane//PSZ)*8*PSZ + c*PSZ + lane%PSZ)
    lane_of_node = np.zeros(N, dtype=np.int64)
    node_of_lane = np.full((CORES, NPADC), -1, dtype=np.int64)
    for c in range(CORES):
        degs_c = deg[c * NPC:(c + 1) * NPC]
        o = np.argsort(-degs_c, kind="stable")
        lane_of_node[c * NPC + o] = np.arange(NPC)
        node_of_lane[c, :NPC] = c * NPC + o
    lane_all = lane_of_node.copy()
    core_all = np.repeat(np.arange(CORES), NPC)
    gpos = (lane_all // PSZ) * CORES * PSZ + core_all * PSZ + (lane_all % PSZ)

    # per-core non-self edge lists sorted by (lane, side-category)
    per_core_edges = []
    mA_all = np.zeros((CORES, NPADC), dtype=np.int64)
    mB_all = np.zeros((CORES, NPADC), dtype=np.int64)
    cnt_all = np.zeros((CORES, NPADC), dtype=np.int64)
    for c in range(CORES):
        sel = (col >= c * NPC) & (col < (c + 1) * NPC)
        lanes = lane_of_node[col[sel]]
        gp = gpos[row[sel]]
        cat = np.ones(len(gp), dtype=np.int64)          # free
        cat[gp < BOFF] = 0                              # must-A
        cat[gp >= WIN] = 2                              # must-B
        sidx = np.lexsort((cat, lanes))
        lanes, gp, cat = lanes[sidx], gp[sidx], cat[sidx]
        cnt = np.bincount(lanes, minlength=NPADC)
        off = np.zeros(NPADC + 1, dtype=np.int64)
        off[1:] = np.cumsum(cnt)
        mA_all[c] = np.bincount(lanes[cat == 0], minlength=NPADC)
        mB_all[c] = np.bincount(lanes[cat == 2], minlength=NPADC)
        cnt_all[c] = cnt
        per_core_edges.append((lanes, gp, off, cnt))

    # joint per-block side capacities: S_A + S_B ~ max block degree, with the
    # per-lane must counts respected; the window overlap absorbs the rest
    D_b = cnt_all.reshape(CORES, BLK, 128).max(axis=(0, 2))
    MA_b = mA_all.reshape(CORES, BLK, 128).max(axis=(0, 2))
    MB_b = mB_all.reshape(CORES, BLK, 128).max(axis=(0, 2))
    SA = np.maximum(np.maximum((D_b + 1) // 2, MA_b), 1)
    SB = np.maximum(np.maximum(D_b - SA, MB_b), 1)
    blk_of_lane = np.arange(NPADC) // 128
    nA_all = np.zeros((CORES, NPADC), dtype=np.int64)
    nB_all = np.zeros((CORES, NPADC), dtype=np.int64)
    for c in range(CORES):
        cnt, mA, mB = cnt_all[c], mA_all[c], mB_all[c]
        lo = np.maximum(mA, cnt - SB[blk_of_lane])
        hi = np.minimum(SA[blk_of_lane], cnt - mB)
        assert (lo <= hi).all()
        nA = np.clip((cnt + 1) // 2, lo, hi)
        nB = cnt - nA
        real = cnt > 0
        bad = real & ((nA == 0) | (nB == 0))
        assert not bad.any(), "dest with an unpopulatable gather side"
        nA_all[c], nB_all[c] = nA, nB
    PA = np.zeros(BLK + 1, dtype=np.int64)
    PA[1:] = np.cumsum(128 * SA)
    PB = np.zeros(BLK + 1, dtype=np.int64)
    PB[1:] = np.cumsum(128 * SB)
    colsA, colsB = int(PA[-1]), int(PB[-1])

    per_core = []
    for c in range(CORES):
        lanes, gp, off, cnt = per_core_edges[c]
        nA, nB = nA_all[c], nB_all[c]
        blk = np.arange(NPADC) // 128
        lane_in_blk = np.arange(NPADC) % 128
        baseA = PA[blk] + lane_in_blk * SA[blk]
        baseB = PB[blk] + lane_in_blk * SB[blk]

        rank = np.arange(len(lanes)) - off[lanes]
        isA = rank < nA[lanes]
        posA = baseA[lanes] + rank
        posB = baseB[lanes] + (rank - nA[lanes])
        tokA_real = gp[isA]
        tokB_real = gp[~isA] - BOFF
        assert len(tokA_real) == 0 or (0 <= tokA_real.min() and tokA_real.max() < WIN)
        assert len(tokB_real) == 0 or (0 <= tokB_real.min() and tokB_real.max() < WIN)

        # slot-0 token per lane (pads duplicate it); 0 for empty lanes
        tok0A = np.zeros(NPADC, dtype=np.int64)
        tok0A[lanes[isA & (rank == 0)]] = gp[isA & (rank == 0)]
        tok0B = np.zeros(NPADC, dtype=np.int64)
        firstB = (~isA) & (rank == nA[lanes])
        tok0B[lanes[firstB]] = gp[firstB] - BOFF

        edA = np.zeros(colsA, dtype=np.int64)
        edB = np.zeros(colsB, dtype=np.int64)
        for b in range(BLK):
            lv = slice(b * 128, (b + 1) * 128)
            edA[PA[b]:PA[b + 1]] = np.repeat(tok0A[lv], SA[b])
            edB[PB[b]:PB[b + 1]] = np.repeat(tok0B[lv], SB[b])
        edA[posA[isA]] = tokA_real
        edB[posB[~isA]] = tokB_real

        npadA = (SA[blk] - nA).astype(np.float64)
        npadB = (SB[blk] - nB).astype(np.float64)

        nodes = node_of_lane[c]
        real = nodes >= 0
        gl = np.where(real, nodes, 0)
        xp = np.zeros((NPADC, D), dtype=np.float32)
        xp[real] = np.asarray(x)[gl[real]]
        xT = np.ascontiguousarray(xp.T)
        dinv_l = np.where(real, dinv[gl], 0.0)
        ddeg_l = np.where(real, ddeg[gl], 0.0)

        per_core.append(dict(
            xT=xT,
            dinv_scale=np.ascontiguousarray(
                dinv_l.reshape(BLK, 128).T).astype(np.float32),
            dinvb=np.broadcast_to(dinv_l, (128, NPADC)).astype(np.float32).copy(),
            ddegb=np.broadcast_to(ddeg_l, (128, NPADC)).astype(np.float32).copy(),
            npadbA=np.broadcast_to(npadA, (128, NPADC)).astype(np.float32).copy(),
            npadbB=np.broadcast_to(npadB, (128, NPADC)).astype(np.float32).copy(),
            eidxA=_wrap_idx(edA), eidxB=_wrap_idx(edB),
            real=real, gl=gl,
        ))
    meta = dict(SA=SA, SB=SB, PA=PA, PB=PB, colsA=colsA, colsB=colsB)
    return per_core, meta


def _chunks(S, P, max_cols):
    out, cur, cur_cols = [], [], 0
    for b in range(BLK):
        w = 128 * int(S[b])
        if cur and cur_cols + w > max_cols:
            out.append(cur)
            cur, cur_cols = [], 0
        cur.append(b)
        cur_cols += w
    if cur:
        out.append(cur)
    return out


def _build_program(meta):
    SA, SB, PA, PB = meta["SA"], meta["SB"], meta["PA"], meta["PB"]
    colsA, colsB = meta["colsA"], meta["colsB"]
    f32, bf16, i16 = mybir.dt.float32, mybir.dt.bfloat16, mybir.dt.int16
    AX = mybir.AxisListType.X
    OP = mybir.AluOpType
    AF = mybir.ActivationFunctionType

    nc = bacc.Bacc("TRN2", target_bir_lowering=False, debug=False,
                   num_devices=CORES)
    t_xT = nc.dram_tensor("xT", [128, NPADC], f32, kind="ExternalInput")
    t_w = [nc.dram_tensor(f"W{l}T", [128, 128], f32 if l == 0 else bf16,
                         kind="ExternalInput") for l in range(2)]
    t_c = [nc.dram_tensor(f"C{l}T", [4, 128, 128], bf16, kind="ExternalInput") for l in range(2)]
    t_b = [nc.dram_tensor(f"b{l}", [128, 1], f32, kind="ExternalInput") for l in range(2)]
    t_wout = nc.dram_tensor("WoutT", [128, NCLS], bf16, kind="ExternalInput")
    t_bout4 = nc.dram_tensor("bout4", [128, 4, NCLS], f32, kind="ExternalInput")
    t_dsc = nc.dram_tensor("dinv_scale", [128, BLK], f32, kind="ExternalInput")
    t_dinvb = nc.dram_tensor("dinvb", [128, NPADC], bf16, kind="ExternalInput")
    t_ddegb = nc.dram_tensor("ddegb", [128, NPADC], bf16, kind="ExternalInput")
    t_npadA = nc.dram_tensor("npadbA", [128, NPADC], bf16, kind="ExternalInput")
    t_npadB = nc.dram_tensor("npadbB", [128, NPADC], bf16, kind="ExternalInput")
    t_eidxA = nc.dram_tensor("eidxA", [128, colsA // 16], i16, kind="ExternalInput")
    t_eidxB = nc.dram_tensor("eidxB", [128, colsB // 16], i16, kind="ExternalInput")
    t_out = nc.dram_tensor("out", [NPADC, NCLS], f32, kind="ExternalOutput")
    t_gsh = [nc.dram_tensor(f"gsh{p}", [PSZ, D], bf16, kind="Internal")
             for p in range(PIECES)]
    t_gfull = [nc.dram_tensor(f"gfull{l}", [NG, D], bf16, kind="Internal")
               for l in range(2)]

    chA = _chunks(SA, PA, MSG_COLS)
    chB = _chunks(SB, PB, MSG_COLS)
    # merged stream: interleave sides ordered by last covered block
    merged = sorted(
        [("A", ch) for ch in chA] + [("B", ch) for ch in chB],
        key=lambda sc: (sc[1][-1], sc[0]))

    with tile.TileContext(nc) as tc, ExitStack() as ctx:
        sb = ctx.enter_context(tc.tile_pool(name="sb", bufs=1))
        lhsp = ctx.enter_context(tc.tile_pool(name="lhsp", bufs=3))
        msgp = ctx.enter_context(tc.tile_pool(name="msgp", bufs=3))
        rhp = ctx.enter_context(tc.tile_pool(name="rhp", bufs=2))
        pg = ctx.enter_context(tc.tile_pool(name="pg", bufs=2, space="PSUM"))
        pc = ctx.enter_context(tc.tile_pool(name="pc", bufs=2, space="PSUM"))
        plg = ctx.enter_context(tc.tile_pool(name="plg", bufs=2, space="PSUM"))

        hT = sb.tile([128, NPADC], bf16, tag="hT")
        gT = sb.tile([128, NPADC], bf16, tag="gT")
        dsc = sb.tile([128, BLK], f32, tag="dsc")
        dinvb = sb.tile([128, NPADC], bf16, tag="dinvb")
        ddegb = sb.tile([128, NPADC], bf16, tag="ddegb")
        npadA = sb.tile([128, NPADC], bf16, tag="npadA")
        npadB = sb.tile([128, NPADC], bf16, tag="npadB")
        eixA = sb.tile([128, colsA // 16], i16, tag="eixA")
        eixB = sb.tile([128, colsB // 16], i16, tag="eixB")
        wout = sb.tile([128, NCLS], bf16, tag="wout")
        bout4 = sb.tile([128, 4, NCLS], f32, tag="bout4")
        wts, cts, bts = [], [], []
        for l in range(2):
            wts.append(sb.tile([128, 128], f32 if l == 0 else bf16,
                                tag=f"wt{l}", name=f"wt{l}"))
            cts.append(sb.tile([128, 4, 128], bf16, tag=f"ct{l}", name=f"ct{l}"))
            bts.append(sb.tile([128, 1], f32, tag=f"bt{l}", name=f"bt{l}"))
        nc.sync.dma_start(wts[0][:], t_w[0].ap())
        nc.sync.dma_start(dsc[:], t_dsc.ap())
        nc.sync.dma_start(dinvb[:], t_dinvb.ap())

        stA_add = sb.tile([128, NPADC], f32, tag="stA_add")
        stB_add = sb.tile([128, NPADC], f32, tag="stB_add")
        stA_mn = sb.tile([128, NPADC], bf16, tag="stA_mn")
        stB_mn = sb.tile([128, NPADC], bf16, tag="stB_mn")
        stA_mx = sb.tile([128, NPADC], bf16, tag="stA_mx")
        stB_mx = sb.tile([128, NPADC], bf16, tag="stB_mx")

        def g_wide(l, jw):
            """g for 512 lanes jw*512..: node-major -> gsh piece, plus
            feature-major gT (the self message) via one wide matmul."""
            wsl = slice(jw * 512, (jw + 1) * 512)
            if l == 0:
                lhs = lhsp.tile([128, 512], f32, tag="lhs")
                nc.sync.dma_start(lhs[:], t_xT.ap()[:, wsl])
                lhs_ap = lhs[:]
            else:
                lhs_ap = hT[:, wsl]
            for k in range(4):
                j = jw * 4 + k
                ps = pg.tile([128, 128], f32, tag="ps_g")
                nc.tensor.matmul(ps[:], lhsT=lhs_ap[:, k * 128:(k + 1) * 128],
                                 rhs=wts[l][:], start=True, stop=True)
                gt = lhsp.tile([128, 128], bf16, tag="gt")
                nc.scalar.activation(gt[:], ps[:], AF.Copy, scale=dsc[:, j:j + 1])
                p = j // (PSZ // 128)
                jj = j - p * (PSZ // 128)
                nc.sync.dma_start(
                    t_gsh[p].ap().rearrange("(a p) d -> p a d", p=128)[:, jj, :],
                    gt[:])
            psT = pg.tile([128, 512], f32, tag="ps_gT")
            nc.tensor.matmul(psT[:], lhsT=wts[l][:], rhs=lhs_ap,
                             start=True, stop=True)
            nc.vector.tensor_tensor(out=gT[:, wsl], in0=psT[:],
                                    in1=dinvb[:, wsl], op=OP.mult)

        def fused_group(l, g):
            """combine + scale + E-matmul for lanes [g*GRP,(g+1)*GRP); then
            next-layer g chunks (l==0) or logits (l==1)."""
            gsl = slice(g * GRP, (g + 1) * GRP)
            nc.vector.tensor_tensor(out=stA_add[:, gsl], in0=stA_add[:, gsl],
                                    in1=stB_add[:, gsl], op=OP.add)
            nc.vector.tensor_tensor(out=stA_mn[:, gsl], in0=stA_mn[:, gsl],
                                    in1=stB_mn[:, gsl], op=OP.min)
            nc.vector.tensor_tensor(out=stA_mx[:, gsl], in0=stA_mx[:, gsl],
                                    in1=stB_mx[:, gsl], op=OP.max)
            # fold in the self-loop message (gT)
            nc.vector.tensor_tensor(out=stA_add[:, gsl], in0=stA_add[:, gsl],
                                    in1=gT[:, gsl], op=OP.add)
            nc.vector.tensor_tensor(out=stA_mn[:, gsl], in0=stA_mn[:, gsl],
                                    in1=gT[:, gsl], op=OP.min)
            nc.vector.tensor_tensor(out=stA_mx[:, gsl], in0=stA_mx[:, gsl],
                                    in1=gT[:, gsl], op=OP.max)
            # scale: mean/add from f32 accumulator; mn/mx in place
            mean_g = rhp.tile([128, GRP], bf16, tag="mean_g")
            add_g = rhp.tile([128, GRP], bf16, tag="add_g")
            nc.vector.tensor_tensor(out=mean_g[:], in0=stA_add[:, gsl],
                                    in1=ddegb[:, gsl], op=OP.mult)
            nc.vector.tensor_tensor(out=add_g[:], in0=stA_add[:, gsl],
                                    in1=dinvb[:, gsl], op=OP.mult)
            nc.vector.tensor_tensor(out=stA_mn[:, gsl], in0=stA_mn[:, gsl],
                                    in1=dinvb[:, gsl], op=OP.mult)
            nc.vector.tensor_tensor(out=stA_mx[:, gsl], in0=stA_mx[:, gsl],
                                    in1=dinvb[:, gsl], op=OP.mult)
            psc = pc.tile([128, GRP], f32, tag="ps_cmb")
            for k, st in enumerate((mean_g[:], add_g[:],
                                    stA_mn[:, gsl], stA_mx[:, gsl])):
                nc.tensor.matmul(psc[:], lhsT=cts[l][:, k, :], rhs=st,
                                 start=(k == 0), stop=(k == 3))
            nc.scalar.activation(hT[:, gsl], psc[:], AF.Relu,
                                 bias=bts[l][:], scale=1.0)
            if l == 0:
                g_wide(1, g)
            else:
                logits_group(g)

        def logits_group(q):
            ps4 = plg.tile([128, 4, NCLS], f32, tag="ps_lg")
            for k in range(4):
                j = q * 4 + k
                nc.tensor.matmul(ps4[:, k, :],
                                 lhsT=hT[:, j * 128:(j + 1) * 128],
                                 rhs=wout[:], start=True, stop=True)
            lg4 = lhsp.tile([128, 4, NCLS], f32, tag="lg4")
            nc.vector.tensor_tensor(out=lg4[:], in0=ps4[:], in1=bout4[:],
                                    op=OP.add)
            ex4 = lhsp.tile([128, 4, NCLS], f32, tag="ex4")
            nc.scalar.activation(ex4[:], lg4[:], AF.Exp)
            se4 = lhsp.tile([128, 4], f32, tag="se4")
            nc.vector.tensor_reduce(out=se4[:], in_=ex4[:], axis=AX, op=OP.add)
            ls4 = lhsp.tile([128, 4], f32, tag="ls4")
            nc.scalar.activation(ls4[:], se4[:], AF.Ln)
            for k in range(4):
                nc.vector.tensor_scalar_sub(lg4[:, k, :], lg4[:, k, :],
                                            ls4[:, k:k + 1])
            nc.sync.dma_start(
                t_out.ap().rearrange("(a p) n -> p a n", p=128)[:, 4 * q:4 * q + 4, :],
                lg4[:])

        def ag_piece(p, dst):
            nc.gpsimd.collective_compute(
                "AllGather", OP.bypass, replica_groups=[list(range(CORES))],
                ins=[t_gsh[p].ap()],
                outs=[t_gfull[dst].ap()[p * CORES * PSZ:(p + 1) * CORES * PSZ, :]])

        # ---- layer 0 A-stage from xT, AllGather pieces as they complete
        for p in range(PIECES):
            for jw in range(p * (PSZ // 512), (p + 1) * (PSZ // 512)):
                g_wide(0, jw)
            ag_piece(p, 0)

        # non-critical loads: after the startup A-chain so they don't delay it
        nc.sync.dma_start(eixA[:], t_eidxA.ap())
        nc.sync.dma_start(eixB[:], t_eidxB.ap())
        nc.sync.dma_start(ddegb[:], t_ddegb.ap())
        nc.sync.dma_start(npadA[:], t_npadA.ap())
        nc.sync.dma_start(npadB[:], t_npadB.ap())
        nc.sync.dma_start(wts[1][:], t_w[1].ap())
        for l in range(2):
            nc.sync.dma_start(cts[l][:], t_c[l].ap().rearrange("k p f -> p k f"))
            nc.sync.dma_start(bts[l][:], t_b[l].ap())
        nc.sync.dma_start(wout[:], t_wout.ap())
        nc.sync.dma_start(bout4[:], t_bout4.ap())

        for l in range(2):
            # ---- gathers + reduces, fused groups as blocks complete
            covA = np.zeros(BLK, dtype=bool)
            covB = np.zeros(BLK, dtype=bool)
            next_g = 0
            ag_next = 0
            for side, ch in merged:
                S, P, eix, lo, hi = (
                    (SA, PA, eixA, 0, WIN) if side == "A"
                    else (SB, PB, eixB, BOFF, NG))
                st_add = stA_add if side == "A" else stB_add
                st_mn = stA_mn if side == "A" else stB_mn
                st_mx = stA_mx if side == "A" else stB_mx
                npadS = npadA if side == "A" else npadB
                q0 = int(P[ch[0]])
                qn = int(P[ch[-1] + 1]) - q0
                msg = msgp.tile([128, 1, MSG_COLS], bf16, tag="msg")
                nc.gpsimd.dma_gather(
                    out_ap=msg[:, :, :qn],
                    in_ap=t_gfull[l].ap()[lo:hi, :],
                    idxs_ap=eix[:, q0 // 16:(q0 + qn) // 16],
                    num_idxs=qn, num_idxs_reg=qn, elem_size=D,
                    transpose=True, single_packet=False)
                for b in ch:
                    sbl = int(S[b])
                    cb = int(P[b]) - q0
                    view = msg[:, 0, cb:cb + 128 * sbl].rearrange(
                        "p (l s) -> p l s", s=sbl)
                    dsl = slice(b * 128, (b + 1) * 128)
                    nc.vector.tensor_reduce(
                        out=st_add[:, dsl], in_=view, axis=AX, op=OP.add)
                    nc.vector.tensor_reduce(
                        out=st_mn[:, dsl], in_=view, axis=AX, op=OP.min)
                    nc.vector.tensor_reduce(
                        out=st_mx[:, dsl], in_=view, axis=AX, op=OP.max)
                    tmp = lhsp.tile([128, 128], f32, tag="tmp")
                    nc.vector.tensor_tensor(
                        out=tmp[:], in0=view[:, :, 0], in1=npadS[:, dsl],
                        op=OP.mult)
                    nc.vector.tensor_tensor(
                        out=st_add[:, dsl], in0=st_add[:, dsl],
                        in1=tmp[:], op=OP.subtract)
                    if side == "A":
                        covA[b] = True
                    else:
                        covB[b] = True
                while next_g < NGRP and covA[next_g * 4:(next_g + 1) * 4].all() \
                        and covB[next_g * 4:(next_g + 1) * 4].all():
                    fused_group(l, next_g)
                    next_g += 1
                # fire layer-1 AllGather pieces once their lanes (+1 group
                # of slack so the Pool never stalls on them) are through E/A
                if l == 0:
                    while ag_next < PIECES - 1 and next_g >= (
                            ((ag_next + 1) * PSZ + GRP - 1) // GRP + 1):
                        ag_piece(ag_next, 1)
                        ag_next += 1
            assert next_g == NGRP
            if l == 0:
                for p in range(ag_next, PIECES):
                    ag_piece(p, 1)

    nc.compile()
    return nc


_CACHE = {}


def kernel(x, edge_index, W0, C0, b0, W1, C1, b1, Wout, bout,
           trace=False, _want_results=False):
    x = np.asarray(x, dtype=np.float32)
    per_core, meta = _host_prep(x, edge_index)
    key = (tuple(meta["SA"]), tuple(meta["SB"]))
    if key not in _CACHE:
        _CACHE[key] = _build_program(meta)
    nc = _CACHE[key]

    import ml_dtypes
    shared = dict(
        W0T=np.ascontiguousarray(np.asarray(W0, np.float32).T),
        W1T=np.ascontiguousarray(np.asarray(W1, np.float32).T).astype(ml_dtypes.bfloat16),
        C0T=np.ascontiguousarray(np.asarray(C0, np.float32).T).reshape(4, 128, 128).astype(ml_dtypes.bfloat16),
        C1T=np.ascontiguousarray(np.asarray(C1, np.float32).T).reshape(4, 128, 128).astype(ml_dtypes.bfloat16),
        b0=np.asarray(b0, np.float32).reshape(128, 1),
        b1=np.asarray(b1, np.float32).reshape(128, 1),
        WoutT=np.ascontiguousarray(np.asarray(Wout, np.float32).T).astype(ml_dtypes.bfloat16),
        bout4=np.broadcast_to(np.asarray(bout, np.float32), (128, 4, NCLS)).copy(),
    )
    in_maps = []
    for c in range(CORES):
        d = per_core[c]
        m = dict(shared)
        m.update(xT=d["xT"], dinv_scale=d["dinv_scale"],
                 dinvb=d["dinvb"].astype(ml_dtypes.bfloat16),
                 ddegb=d["ddegb"].astype(ml_dtypes.bfloat16),
                 npadbA=d["npadbA"].astype(ml_dtypes.bfloat16),
                 npadbB=d["npadbB"].astype(ml_dtypes.bfloat16),
                 eidxA=d["eidxA"], eidxB=d["eidxB"])
        in_maps.append(m)

    res = bass_utils.run_bass_kernel_spmd(
        nc, in_maps, core_ids=list(range(CORES)), trace=trace)

    out = np.zeros((N, NCLS), dtype=np.float32)
    for c in range(CORES):
        o = res.results[c]["out"]
        d = per_core[c]
        out[d["gl"][d["real"]]] = o[d["real"]]
    if _want_results:
        return out, res
    return out
